# revision 25
# baseline (speedup 1.0000x reference)
"""Tensor-parallel multi-head attention kernel for 8 Trainium2 NeuronCores.

Problem: nn_Attention (B=2, S=2048, D=2048, 16 heads x 128) with per-head
RMSNorm on q/k, non-causal softmax attention, and output projection.

Sharding (tensor-parallel over heads, per the hint):
  - core c owns heads {2c, 2c+1}: Wq/Wk/Wv column slices [D, 256], Wo row
    slice [256, D].
  - every core reads all of x (the projection contracts over full D and
    full sequence is needed for non-causal attention keys/values).
  - each core emits a partial output  out_c = attn_out_c @ Wo_c ; the
    host unshard sums the 8 partials (the natural unshard for row-sharded
    Wo -- equivalent to the all-reduce in the hint, done at gather time).

Pipeline structure (v3):
  - fp16 everywhere device-side; host pre-tiles inputs and sums fp16
    partials in f32.  Matmul rate is unchanged vs f32r but LDWEIGHTS
    runs 2+ elem/cycle (FWL), DMA bytes halve, DVE gets its 2x mode.
  - softmax denominator via a DVE pair-sum tree (fp16 2x) + a single
    accumulating ones-matmul; the old 16 ones-matmuls were 60us of PE.
  - the attention kj loop is ACT-exp paced (~610ns/exp vs 466ns of PE
    work) and the PE executes matmuls in program order, so all slack
    work is PUMPED into the kj loop: the previous slot's den/recip
    chain (kj 1-4), the previous query block's output projection (odd
    kj), and -- during batch-0 attention -- batch-1's entire
    projection+rmsnorm phase, one pass per pump point (kj 5,7,9,11,13).
    This keeps the PE saturated through the ACT-bound attention phase
    and removes the batch boundary entirely (qkv tiles double-buffer).
  - PSUM banks (8): attn scores + proj ssum ring 2, proj q/k ring 2,
    attn av ring 2, {outproj, v-proj, den} ring 2.
  - startup: first matmul depends only on the first wq + xt chunks;
    DMA order is wq, xt(chunk0), wk, wv, xt(chunk1), ... so the PE
    starts as soon as ~1MB has landed instead of after all weights.
"""

import math
import sys

for _p in ("/opt/trn_rl_repo",):
    if _p not in sys.path:
        sys.path.insert(0, _p)

import numpy as np

import bass_rust
import concourse.bass as bass
import concourse.mybir as mybir
import concourse.tile as tile

F32 = mybir.dt.float32
F16 = mybir.dt.float16
AF = mybir.ActivationFunctionType
MUL = mybir.AluOpType.mult

N_CORES = 8
N_HEADS = 16
HEAD_DIM = 128
EPS = 1e-6
TREE_LVLS = 4      # levels of DVE pair-summing before the ones-matmul
PROJ_PUMP_KJ = (5, 7, 9, 11, 13)   # proj-unit pump points in the kj loop

_wait_counter = [0]


def _split_waits(nc, limit=1):
    """This compiler build rejects >1 semaphore wait per instruction
    ("Too many sync wait commands").  Move excess waits onto preceding
    same-engine no-ops: the sequencer executes them in order, so waiting
    earlier on the same engine is semantically equivalent."""
    for fn in nc.m.functions:
        for blk in fn.blocks:
            newl = []
            changed = False
            for inst in blk.instructions:
                si = inst.sync_info
                waits = list(si.on_wait) if si is not None and si.on_wait else []
                if len(waits) > limit:
                    extra, keep = waits[:-limit], waits[-limit:]
                    for w in extra:
                        _wait_counter[0] += 1
                        nop = bass_rust.InstNoOp(name=f"I-waitsplit-{_wait_counter[0]}")
                        nop.engine = inst.engine
                        nop.sync_info = mybir.SyncInfo(on_wait=[w], on_update=[])
                        newl.append(nop)
                    si.on_wait = keep
                    changed = True
                newl.append(inst)
            if changed:
                blk.instructions = newl


def build_nc(B, S, D, HL, split=True):
    """Emit the per-core program. HL = heads per core."""
    IL = HL * HEAD_DIM          # local inner dim
    NKB = D // 128              # contraction blocks for projections
    SC = 512                    # seq chunk for the projection phase
    NSC = S // SC
    NQB = S // 512              # query blocks in attention
    NKJ = S // 128              # key blocks in attention
    BS = B * S
    scale = 1.0 / math.sqrt(HEAD_DIM)

    nc = bass.Bass("TRN2", target_bir_lowering=False, debug=False,
                   num_devices=N_CORES)
    # host-pre-tiled layouts: [partition, kb, free]
    xt_d = nc.dram_tensor("xt_d", [128, NKB, BS], F16, kind="ExternalInput")
    Wq = nc.dram_tensor("Wq", [128, NKB, IL], F16, kind="ExternalInput")
    Wk = nc.dram_tensor("Wk", [128, NKB, IL], F16, kind="ExternalInput")
    Wv = nc.dram_tensor("Wv", [128, NKB, IL], F16, kind="ExternalInput")
    Wo = nc.dram_tensor("Wo", [128, HL, D], F16, kind="ExternalInput")
    qg = nc.dram_tensor("qg", [128, 1], F32, kind="ExternalInput")
    kg = nc.dram_tensor("kg", [128, 1], F32, kind="ExternalInput")
    out = nc.dram_tensor("out", [BS, D], F16, kind="ExternalOutput")

    with tile.TileContext(nc) as tc:
        with (
            tc.tile_pool(name="wpool", bufs=1) as wpool,
            tc.tile_pool(name="xpool", bufs=2) as xpool,
            tc.tile_pool(name="x0q", bufs=1) as x0q,
            tc.tile_pool(name="qkv", bufs=2) as qkv,
            tc.tile_pool(name="otp", bufs=5) as otp,
            tc.tile_pool(name="tmp", bufs=2) as tmp,
            tc.tile_pool(name="sqp", bufs=4) as sqp,
            tc.tile_pool(name="expool", bufs=8) as expool,
            tc.tile_pool(name="trp", bufs=4) as trp,
            tc.tile_pool(name="oop", bufs=10) as oop,
            tc.tile_pool(name="ps", bufs=1, space="PSUM") as ps,
        ):
            # ---- resident constants / weights ----
            wq_sb = wpool.tile([128, NKB, IL], F16, name="wq_sb")
            for kb4 in range(NKB // 4):
                nc.sync.dma_start(wq_sb[:, kb4 * 4:(kb4 + 1) * 4, :],
                                  Wq[:, kb4 * 4:(kb4 + 1) * 4, :])
            gq = wpool.tile([128, 1], F32, name="gq")
            nc.sync.dma_start(gq[:], qg[:])
            gk = wpool.tile([128, 1], F32, name="gk")
            nc.sync.dma_start(gk[:], kg[:])
            wk_sb = wpool.tile([128, NKB, IL], F16, name="wk_sb")
            wv_sb = wpool.tile([128, NKB, IL], F16, name="wv_sb")
            wo_sb = wpool.tile([128, HL, D], F16, name="wo_sb")
            ones_f32 = wpool.tile([128, 128], F32, name="ones_f32")
            nc.vector.memset(ones_f32[:], 1.0)
            ones16 = wpool.tile([128, 128], F16, name="ones16")
            nc.scalar.copy(ones16[:], ones_f32[:])
            eps_sb = wpool.tile([128, 1], F32, name="eps_sb")
            nc.vector.memset(eps_sb[:], EPS)
            # HAM warmup: the PE cold-throttles (K=4/8, ~2x slow) for the
            # first ~4us of matmuls.  Run dummy ones-matmuls while the
            # startup DMAs are in flight so real matmuls start at full
            # rate.  No readers -> the po-ring slots recycle instantly.
            for wu in range(60):
                wu_ps = ps.tile([128, 128], F32, tag="po", bufs=2,
                                name=f"wu{wu}")
                nc.tensor.matmul(wu_ps[:], ones16[:], ones16[:],
                                 start=True, stop=True)

            # ================= emission helpers =================
            # qkv tiles per batch (ring 2 => batches coexist)
            qkv_t = {}

            def ensure_qkv(b):
                if b not in qkv_t:
                    qkv_t[b] = (
                        qkv.tile([128, HL, S], F16, tag="qt", name=f"qt{b}"),
                        qkv.tile([128, HL, S], F16, tag="kt", name=f"kt{b}"),
                        qkv.tile([128, NKJ, IL], F16, tag="vt", name=f"vt{b}"),
                    )
                return qkv_t[b]

            xt_t = {}

            def emit_xt_dma(b, sc, split_rings=False):
                off = b * S + sc * SC
                if split_rings:
                    # startup chunk: four independent quarter tiles on the
                    # scalar HWDGE ring (weights stream on the sync ring in
                    # parallel), so the first matmul waits only for the
                    # first 512KB quarter instead of the whole 2MB chunk.
                    quads = []
                    for kb4 in range(NKB // 4):
                        q = x0q.tile([128, 4, SC], F16, tag=f"x0q{kb4}",
                                     name=f"x0q_{kb4}")
                        nc.scalar.dma_start(
                            q[:], xt_d[:, kb4 * 4:(kb4 + 1) * 4,
                                       off:off + SC])
                        quads.append(q)
                    xt_t[(b, sc)] = ("quad", quads)
                    return
                xt = xpool.tile([128, NKB, SC], F16, tag="xt",
                                name=f"xt{b}_{sc}")
                xt_t[(b, sc)] = xt
                for kb4 in range(NKB // 4):
                    nc.sync.dma_start(
                        xt[:, kb4 * 4:(kb4 + 1) * 4, :],
                        xt_d[:, kb4 * 4:(kb4 + 1) * 4, off:off + SC])

            def xt_slice(b, sc, kb, cols=None):
                xt = xt_t[(b, sc)]
                if isinstance(xt, tuple):
                    q = xt[1][kb // 4]
                    s = q[:, kb % 4, :]
                else:
                    s = xt[:, kb, :]
                return s if cols is None else s[:, cols]

            def emit_qk_pass(b, sc, pi, h):
                qt, kt, vt = ensure_qkv(b)
                w_sb, dstT, gam, pname = (
                    (wq_sb, qt, gq, "q") if pi == 0 else (wk_sb, kt, gk, "k"))
                ph = ps.tile([128, SC], F32, tag="ph", bufs=2,
                             name=f"ph_{pname}{b}_{sc}_{h}")
                for kb in range(NKB):
                    nc.tensor.matmul(
                        ph[:], w_sb[:, kb, h * 128:(h + 1) * 128],
                        xt_slice(b, sc, kb),
                        start=(kb == 0), stop=(kb == NKB - 1))
                sq = sqp.tile([128, SC], F16, tag="sq",
                              name=f"sq_{pname}{b}_{sc}_{h}")
                nc.scalar.activation(sq[:], ph[:], AF.Square)
                ssum = ps.tile([128, SC], F32, tag="sc", bufs=2,
                               name=f"ssum_{pname}{b}_{sc}_{h}")
                nc.tensor.matmul(ssum[:], ones16[:], sq[:],
                                 start=True, stop=True)
                # rstd = (mean(x^2)+eps)^-0.5 = exp(-0.5*ln(.))
                lg = tmp.tile([128, SC], F32, tag="lg",
                              name=f"lg_{pname}{b}_{sc}_{h}")
                nc.scalar.activation(lg[:], ssum[:], AF.Ln,
                                     bias=eps_sb[:], scale=1.0 / HEAD_DIM)
                rstd = tmp.tile([128, SC], F32, tag="rstd",
                                name=f"rstd_{pname}{b}_{sc}_{h}")
                nc.scalar.activation(rstd[:], lg[:], AF.Exp, scale=-0.5)
                nc.vector.scalar_tensor_tensor(
                    out=dstT[:, h, sc * SC:(sc + 1) * SC],
                    in0=ph[:], scalar=gam[:], in1=rstd[:],
                    op0=MUL, op1=MUL)

            def emit_v_pass(b, sc, j):
                _, _, vt = ensure_qkv(b)
                pv = ps.tile([128, IL], F32, tag="po", bufs=2,
                             name=f"pv{b}_{sc}_{j}")
                for kb in range(NKB):
                    nc.tensor.matmul(
                        pv[:], xt_slice(b, sc, kb,
                                        slice(j * 128, (j + 1) * 128)),
                        wv_sb[:, kb, :],
                        start=(kb == 0), stop=(kb == NKB - 1))
                nc.vector.tensor_copy(vt[:, sc * (SC // 128) + j, :], pv[:])

            def emit_wkwv_dma():
                for kb4 in range(NKB // 4):
                    nc.sync.dma_start(wk_sb[:, kb4 * 4:(kb4 + 1) * 4, :],
                                      Wk[:, kb4 * 4:(kb4 + 1) * 4, :])
                for kb4 in range(NKB // 4):
                    nc.sync.dma_start(wv_sb[:, kb4 * 4:(kb4 + 1) * 4, :],
                                      Wv[:, kb4 * 4:(kb4 + 1) * 4, :])

            def make_proj_units(b, first=False, defer_last=False):
                """Ordered emission units for batch b's projection phase.
                DMA for chunk sc+2 is emitted only after every chunk-sc
                unit (the xt ring-2 overwrite dependency tracks only
                already-emitted readers).  With defer_last, the final
                chunk's q and v passes are returned separately: they can
                run during batch b's OWN attention (vt[12..15] is first
                read ~7us into slot 0, qt cols 1536+ only at qb3), which
                moves PE work from the PE-bound merged phase into the
                ACT-bound final attention phase."""
                units = []
                units.append(lambda b=b: emit_xt_dma(b, 0, split_rings=first))
                if first:
                    units.append(emit_wkwv_dma)
                units.append(lambda b=b: emit_xt_dma(b, 1))
                late_v = []
                for sc in range(NSC):
                    defer = defer_last and sc == NSC - 1
                    for pi in range(2):
                        for h in range(HL):
                            units.append(
                                lambda b=b, sc=sc, pi=pi, h=h:
                                emit_qk_pass(b, sc, pi, h))
                    for j in range(SC // 128):
                        u = (lambda b=b, sc=sc, j=j: emit_v_pass(b, sc, j))
                        (late_v if defer else units).append(u)
                    if sc + 2 < NSC:
                        units.append(
                            lambda b=b, sc=sc: emit_xt_dma(b, sc + 2))
                # vt[12..15] is read from kj 12 of the very first
                # attention slot, so the deferred v passes pump early.
                return units, late_v

            # ---- attention software-pipeline state ----
            pend_tail = [None]
            pend_po = [None]
            cur_attn_b = [0]
            po_units = []
            proj_units = []
            late_units = []

            def tail_step(step):
                """One stage of the previous slot's softmax tail.
                step 1: den matmul; 2: Ln; 3: recip exp; 4: ot mul."""
                if pend_tail[0] is None:
                    return
                av, roots, ot_t, h, bb, qq, st = pend_tail[0]
                if step == 1:
                    den = ps.tile([128, 512], F32, tag="po", bufs=2,
                                  name=f"den{bb}_{qq}_{h}")
                    nr = len(roots)
                    for i, rt in enumerate(roots):
                        nc.tensor.matmul(den[:], ones16[:], rt[:],
                                         start=(i == 0), stop=(i == nr - 1))
                    st["den"] = den
                elif step == 2:
                    lden = tmp.tile([128, 512], F32, tag="lden",
                                    name=f"lden{bb}_{qq}_{h}")
                    nc.scalar.activation(lden[:], st["den"][:], AF.Ln)
                    st["lden"] = lden
                elif step == 3:
                    r = tmp.tile([128, 512], F32, tag="r",
                                 name=f"r{bb}_{qq}_{h}")
                    nc.scalar.activation(r[:], st["lden"][:], AF.Exp,
                                         scale=-1.0)
                    st["r"] = r
                elif step == 4:
                    nc.vector.tensor_mul(ot_t[:, h, :], av[:], st["r"][:])
                    pend_tail[0] = None

            def po_load_units():
                if pend_po[0] is None:
                    return
                bb, qq, ot_t = pend_po[0]
                pend_po[0] = None
                for qs in range(4):
                    for dc in range(D // 512):
                        po_units.append((bb, qq, ot_t, qs, dc))

            def po_step(n=1, split_rings=False):
                """Emit n output-projection units (2 matmuls + copy + DMA).
                split_rings alternates the out-DMA between the sync and
                scalar HWDGE rings -- only safe when the scalar (ACT)
                queue has no pending exp work, i.e. at the final flush."""
                for i_ in range(n):
                    if not po_units:
                        return
                    bb, qq, ot_t, qs, dc = po_units.pop(0)
                    qi = qq * 4 + qs
                    po = ps.tile([128, 512], F32, tag="po", bufs=2,
                                 name=f"po{bb}_{qi}_{dc}")
                    for h in range(HL):
                        nc.tensor.matmul(
                            po[:], ot_t[:, h, qs * 128:(qs + 1) * 128],
                            wo_sb[:, h, dc * 512:(dc + 1) * 512],
                            start=(h == 0), stop=(h == HL - 1))
                    oo = oop.tile([128, 512], F16, tag="oo",
                                  name=f"oo{bb}_{qi}_{dc}")
                    nc.vector.tensor_copy(oo[:], po[:])
                    eng = nc.scalar if (split_rings and i_ % 2 == 1) else nc.sync
                    eng.dma_start(
                        out[bb * S + qi * 128: bb * S + (qi + 1) * 128,
                            dc * 512:(dc + 1) * 512], oo[:])

            def proj_step(n=1):
                for _ in range(n):
                    if not proj_units:
                        return
                    proj_units.pop(0)()

            def late_step(n=1):
                for _ in range(n):
                    if not late_units:
                        return
                    late_units.pop(0)()

            # ================= main schedule =================
            # batch-0 projection runs pure (nothing to overlap with).
            u0, _ = make_proj_units(0, first=True)
            for u in u0:
                u()
            nc.sync.dma_start(wo_sb[:], Wo[:])
            u1, l1 = make_proj_units(1, defer_last=True)
            proj_units.extend(u1)
            late_units.extend(l1)

            for b in range(B):
                cur_attn_b[0] = b
                if b > 0:
                    # safety: attention for b reads qt/kt/vt(b) -- every
                    # proj unit must be emitted by now (normally the
                    # pump points have already drained the list).
                    proj_step(len(proj_units))
                qt, kt, vt = ensure_qkv(b)
                for qb in range(NQB):
                    ot_qb = otp.tile([128, HL, 512], F16, tag="ot",
                                     name=f"ot{b}_{qb}")
                    for h in range(HL):
                        if h == HL - 1:
                            po_load_units()
                        av = ps.tile([128, 512], F32, tag="av", bufs=2,
                                     name=f"av{b}_{h}_{qb}")
                        roots = []
                        lvl_pend = [[] for _ in range(TREE_LVLS)]
                        exs = []

                        def av_and_tree(j, h=h, av=av, exs=exs,
                                        lvl_pend=lvl_pend, roots=roots,
                                        b=b, qb=qb):
                            # av matmul + pair-sum tree for key block j;
                            # runs one kj behind the score matmuls so the
                            # in-order PE never stalls on exp(j): the
                            # sc->exp->av latency chain paced the pure
                            # attention phase at ~19us/slot.
                            nc.tensor.matmul(
                                av[:], vt[:, j, h * 128:(h + 1) * 128],
                                exs[j][:],
                                start=(j == 0), stop=(j == NKJ - 1))
                            node = exs[j]
                            for lv in range(TREE_LVLS):
                                lvl_pend[lv].append(node)
                                if len(lvl_pend[lv]) < 2:
                                    node = None
                                    break
                                a0, a1 = lvl_pend[lv]
                                lvl_pend[lv] = []
                                t = trp.tile([128, 512], F16, tag=f"tr{lv}",
                                             bufs=(4 if lv < 3 else 3),
                                             name=f"tr{lv}_{b}_{h}_{qb}_{j}")
                                nc.vector.tensor_add(t[:], a0[:], a1[:])
                                node = t
                            if node is not None:
                                roots.append(node)

                        for kj in range(NKJ):
                            # in the final batch's attention nothing uses
                            # the ph ring, so alternate the score psum
                            # between sc and ph tags: effective ring 4.
                            stag = "ph" if (b == B - 1 and kj % 2 == 1) \
                                else "sc"
                            sc_ps = ps.tile([128, 512], F32, tag=stag,
                                            bufs=2,
                                            name=f"sc{b}_{h}_{qb}_{kj}")
                            nc.tensor.matmul(
                                sc_ps[:], kt[:, h, kj * 128:(kj + 1) * 128],
                                qt[:, h, qb * 512:(qb + 1) * 512],
                                start=True, stop=True)
                            ex = expool.tile([128, 512], F16, tag="ex",
                                             name=f"ex{b}_{h}_{qb}_{kj}")
                            nc.scalar.activation(ex[:], sc_ps[:], AF.Exp,
                                                 scale=scale)
                            exs.append(ex)
                            # lookahead depth: 2 in the final batch (its
                            # sc+ph alternation gives an effective ring
                            # of 4), 1 elsewhere (sc ring is only 2).
                            lag = 2 if b == B - 1 else 1
                            if kj == 0:
                                po_step(1)
                            if kj >= lag:
                                av_and_tree(kj - lag)
                            # pump slack work into the PE's exp-wait gaps
                            # (the PE executes matmuls in program order).
                            if kj in (1, 2, 3, 4):
                                tail_step(kj)
                            if kj % 2 == 1:
                                po_step(1)
                            if kj in PROJ_PUMP_KJ:
                                proj_step(1)
                            if b == B - 1 and kj in (2, 4, 6, 8):
                                # deferred b1 v passes: vt[12..15] is
                                # first read at kj 12 of this slot.
                                late_step(1)
                        for j in range(NKJ - lag, NKJ):
                            av_and_tree(j)
                        pend_tail[0] = (av, roots, ot_qb, h, b, qb, {})
                    # drain toward 2 (keep kj0 gap-fillers) but cap the
                    # burst so the ACT exp pipeline never starves behind
                    # a long run of po matmuls.
                    po_step(min(max(0, len(po_units) - 2), 8))
                    pend_po[0] = (b, qb, ot_qb)

            # final flush
            proj_step(len(proj_units))
            late_step(len(late_units))
            for stp in (1, 2, 3, 4):
                tail_step(stp)
            po_load_units()
            po_step(len(po_units), split_rings=True)

    if split:
        _split_waits(nc)
    return nc


def _prep_in_maps(inputs, B, S, D, HL):
    """Shard + pre-tile the full inputs for the 8 cores (all fp16)."""
    BS = B * S
    NKB = D // 128
    IL = HL * HEAD_DIM
    x = np.asarray(inputs["x"], dtype=np.float32)
    Wq = np.asarray(inputs["Wq"], dtype=np.float32)
    Wk = np.asarray(inputs["Wk"], dtype=np.float32)
    Wv = np.asarray(inputs["Wv"], dtype=np.float32)
    Wo = np.asarray(inputs["Wo"], dtype=np.float32)
    qg = np.ascontiguousarray(
        np.asarray(inputs["q_gamma"], dtype=np.float32).reshape(128, 1))
    kg = np.ascontiguousarray(
        np.asarray(inputs["k_gamma"], dtype=np.float32).reshape(128, 1))
    # x[bs, kb*128+p] -> [p, kb, bs]
    xtld = np.ascontiguousarray(
        x.reshape(BS, NKB, 128).transpose(2, 1, 0).astype(np.float16))
    in_maps = []
    for c in range(N_CORES):
        cs = slice(c * IL, (c + 1) * IL)
        # W[kb*128+p, n] -> [p, kb, n]
        wq_t = np.ascontiguousarray(
            Wq[:, cs].reshape(NKB, 128, IL).transpose(1, 0, 2).astype(np.float16))
        wk_t = np.ascontiguousarray(
            Wk[:, cs].reshape(NKB, 128, IL).transpose(1, 0, 2).astype(np.float16))
        wv_t = np.ascontiguousarray(
            Wv[:, cs].reshape(NKB, 128, IL).transpose(1, 0, 2).astype(np.float16))
        # Wo[h*128+p, :] -> [p, h, :]
        wo_t = np.ascontiguousarray(
            Wo[cs, :].reshape(HL, 128, D).transpose(1, 0, 2).astype(np.float16))
        in_maps.append({
            "xt_d": xtld,
            "Wq": wq_t,
            "Wk": wk_t,
            "Wv": wv_t,
            "Wo": wo_t,
            "qg": qg,
            "kg": kg,
        })
    return in_maps


_NC_CACHE = {}


def run_cores(inputs, trace=False):
    """Build (cached), shard, run on 8 cores; returns (full_out, results)."""
    from concourse.bass_utils import run_bass_kernel_spmd

    x = np.asarray(inputs["x"])
    B, S, D = x.shape
    HL = N_HEADS // N_CORES
    key = (B, S, D, HL)
    if key not in _NC_CACHE:
        _NC_CACHE[key] = build_nc(B, S, D, HL)
    nc = _NC_CACHE[key]
    in_maps = _prep_in_maps(inputs, B, S, D, HL)
    res = run_bass_kernel_spmd(nc, in_maps, list(range(N_CORES)), trace=trace)
    acc = res.results[0]["out"].astype(np.float32)
    for c in range(1, N_CORES):
        acc = acc + res.results[c]["out"].astype(np.float32)
    return acc.reshape(B, S, D), res


def kernel(**inputs) -> np.ndarray:
    return run_cores(inputs, trace=False)[0]


# revision 26
# speedup vs baseline: 1.0083x; 1.0083x over previous
"""Tensor-parallel multi-head attention kernel for 8 Trainium2 NeuronCores.

Problem: nn_Attention (B=2, S=2048, D=2048, 16 heads x 128) with per-head
RMSNorm on q/k, non-causal softmax attention, and output projection.

Sharding (tensor-parallel over heads, per the hint):
  - core c owns heads {2c, 2c+1}: Wq/Wk/Wv column slices [D, 256], Wo row
    slice [256, D].
  - every core reads all of x (the projection contracts over full D and
    full sequence is needed for non-causal attention keys/values).
  - each core emits a partial output  out_c = attn_out_c @ Wo_c ; the
    host unshard sums the 8 partials (the natural unshard for row-sharded
    Wo -- equivalent to the all-reduce in the hint, done at gather time).

Pipeline structure (v3):
  - fp16 everywhere device-side; host pre-tiles inputs and sums fp16
    partials in f32.  Matmul rate is unchanged vs f32r but LDWEIGHTS
    runs 2+ elem/cycle (FWL), DMA bytes halve, DVE gets its 2x mode.
  - softmax denominator via a DVE pair-sum tree (fp16 2x) + a single
    accumulating ones-matmul; the old 16 ones-matmuls were 60us of PE.
  - the attention kj loop is ACT-exp paced (~610ns/exp vs 466ns of PE
    work) and the PE executes matmuls in program order, so all slack
    work is PUMPED into the kj loop: the previous slot's den/recip
    chain (kj 1-4), the previous query block's output projection (odd
    kj), and -- during batch-0 attention -- batch-1's entire
    projection+rmsnorm phase, one pass per pump point (kj 5,7,9,11,13).
    This keeps the PE saturated through the ACT-bound attention phase
    and removes the batch boundary entirely (qkv tiles double-buffer).
  - PSUM banks (8): attn scores + proj ssum ring 2, proj q/k ring 2,
    attn av ring 2, {outproj, v-proj, den} ring 2.
  - startup: first matmul depends only on the first wq + xt chunks;
    DMA order is wq, xt(chunk0), wk, wv, xt(chunk1), ... so the PE
    starts as soon as ~1MB has landed instead of after all weights.
"""

import math
import sys

for _p in ("/opt/trn_rl_repo",):
    if _p not in sys.path:
        sys.path.insert(0, _p)

import numpy as np

import bass_rust
import concourse.bass as bass
import concourse.mybir as mybir
import concourse.tile as tile

F32 = mybir.dt.float32
F16 = mybir.dt.float16
AF = mybir.ActivationFunctionType
MUL = mybir.AluOpType.mult

N_CORES = 8
N_HEADS = 16
HEAD_DIM = 128
EPS = 1e-6
TREE_LVLS = 4      # levels of DVE pair-summing before the ones-matmul
PROJ_PUMP_KJ = (5, 7, 9, 11, 13)   # proj-unit pump points in the kj loop

_wait_counter = [0]


def _split_waits(nc, limit=1):
    """This compiler build rejects >1 semaphore wait per instruction
    ("Too many sync wait commands").  Move excess waits onto preceding
    same-engine no-ops: the sequencer executes them in order, so waiting
    earlier on the same engine is semantically equivalent."""
    for fn in nc.m.functions:
        for blk in fn.blocks:
            newl = []
            changed = False
            for inst in blk.instructions:
                si = inst.sync_info
                waits = list(si.on_wait) if si is not None and si.on_wait else []
                if len(waits) > limit:
                    extra, keep = waits[:-limit], waits[-limit:]
                    for w in extra:
                        _wait_counter[0] += 1
                        nop = bass_rust.InstNoOp(name=f"I-waitsplit-{_wait_counter[0]}")
                        nop.engine = inst.engine
                        nop.sync_info = mybir.SyncInfo(on_wait=[w], on_update=[])
                        newl.append(nop)
                    si.on_wait = keep
                    changed = True
                newl.append(inst)
            if changed:
                blk.instructions = newl


def build_nc(B, S, D, HL, split=True):
    """Emit the per-core program. HL = heads per core."""
    IL = HL * HEAD_DIM          # local inner dim
    NKB = D // 128              # contraction blocks for projections
    SC = 512                    # seq chunk for the projection phase
    NSC = S // SC
    NQB = S // 512              # query blocks in attention
    NKJ = S // 128              # key blocks in attention
    BS = B * S
    scale = 1.0 / math.sqrt(HEAD_DIM)

    nc = bass.Bass("TRN2", target_bir_lowering=False, debug=False,
                   num_devices=N_CORES)
    # host-pre-tiled layouts: [partition, kb, free]
    xt_d = nc.dram_tensor("xt_d", [128, NKB, BS], F16, kind="ExternalInput")
    Wq = nc.dram_tensor("Wq", [128, NKB, IL], F16, kind="ExternalInput")
    Wk = nc.dram_tensor("Wk", [128, NKB, IL], F16, kind="ExternalInput")
    Wv = nc.dram_tensor("Wv", [128, NKB, IL], F16, kind="ExternalInput")
    Wo = nc.dram_tensor("Wo", [128, HL, D], F16, kind="ExternalInput")
    qg = nc.dram_tensor("qg", [128, 1], F32, kind="ExternalInput")
    kg = nc.dram_tensor("kg", [128, 1], F32, kind="ExternalInput")
    out = nc.dram_tensor("out", [BS, D], F16, kind="ExternalOutput")

    with tile.TileContext(nc) as tc:
        with (
            tc.tile_pool(name="wpool", bufs=1) as wpool,
            tc.tile_pool(name="xpool", bufs=2) as xpool,
            tc.tile_pool(name="x0q", bufs=1) as x0q,
            tc.tile_pool(name="qkv", bufs=2) as qkv,
            tc.tile_pool(name="otp", bufs=5) as otp,
            tc.tile_pool(name="tmp", bufs=2) as tmp,
            tc.tile_pool(name="sqp", bufs=4) as sqp,
            tc.tile_pool(name="expool", bufs=8) as expool,
            tc.tile_pool(name="trp", bufs=4) as trp,
            tc.tile_pool(name="oop", bufs=10) as oop,
            tc.tile_pool(name="ps", bufs=1, space="PSUM") as ps,
        ):
            # ---- resident constants / weights ----
            wq_sb = wpool.tile([128, NKB, IL], F16, name="wq_sb")
            for kb4 in range(NKB // 4):
                nc.sync.dma_start(wq_sb[:, kb4 * 4:(kb4 + 1) * 4, :],
                                  Wq[:, kb4 * 4:(kb4 + 1) * 4, :])
            gq = wpool.tile([128, 1], F32, name="gq")
            nc.sync.dma_start(gq[:], qg[:])
            gk = wpool.tile([128, 1], F32, name="gk")
            nc.sync.dma_start(gk[:], kg[:])
            wk_sb = wpool.tile([128, NKB, IL], F16, name="wk_sb")
            wv_sb = wpool.tile([128, NKB, IL], F16, name="wv_sb")
            wo_sb = wpool.tile([128, HL, D], F16, name="wo_sb")
            ones_f32 = wpool.tile([128, 128], F32, name="ones_f32")
            nc.vector.memset(ones_f32[:], 1.0)
            ones16 = wpool.tile([128, 128], F16, name="ones16")
            nc.scalar.copy(ones16[:], ones_f32[:])
            eps_sb = wpool.tile([128, 1], F32, name="eps_sb")
            nc.vector.memset(eps_sb[:], EPS)
            # HAM warmup: the PE cold-throttles (K=4/8, ~2x slow) for the
            # first ~4us of matmuls.  Run dummy ones-matmuls while the
            # startup DMAs are in flight so real matmuls start at full
            # rate.  No readers -> the po-ring slots recycle instantly.
            for wu in range(60):
                wu_ps = ps.tile([128, 128], F32, tag="po", bufs=2,
                                name=f"wu{wu}")
                nc.tensor.matmul(wu_ps[:], ones16[:], ones16[:],
                                 start=True, stop=True)

            # ================= emission helpers =================
            # qkv tiles per batch (ring 2 => batches coexist)
            qkv_t = {}

            def ensure_qkv(b):
                if b not in qkv_t:
                    qkv_t[b] = (
                        qkv.tile([128, HL, S], F16, tag="qt", name=f"qt{b}"),
                        qkv.tile([128, HL, S], F16, tag="kt", name=f"kt{b}"),
                        qkv.tile([128, NKJ, IL], F16, tag="vt", name=f"vt{b}"),
                    )
                return qkv_t[b]

            xt_t = {}

            def emit_xt_dma(b, sc, split_rings=False, rings2=False):
                off = b * S + sc * SC
                if split_rings:
                    # startup chunk: four independent quarter tiles on the
                    # scalar HWDGE ring (weights stream on the sync ring in
                    # parallel), so the first matmul waits only for the
                    # first 512KB quarter instead of the whole 2MB chunk.
                    quads = []
                    for kb4 in range(NKB // 4):
                        q = x0q.tile([128, 4, SC], F16, tag=f"x0q{kb4}",
                                     name=f"x0q_{kb4}")
                        nc.scalar.dma_start(
                            q[:], xt_d[:, kb4 * 4:(kb4 + 1) * 4,
                                       off:off + SC])
                        quads.append(q)
                    xt_t[(b, sc)] = ("quad", quads)
                    return
                xt = xpool.tile([128, NKB, SC], F16, tag="xt",
                                name=f"xt{b}_{sc}")
                xt_t[(b, sc)] = xt
                for kb4 in range(NKB // 4):
                    # batch-0 chunks stream on both HWDGE rings (the ACT
                    # queue is ~40% busy in pure proj); batch-1 chunks
                    # stay on sync (ACT paces the merged attention).
                    eng = nc.scalar if (rings2 and kb4 < 2) else nc.sync
                    eng.dma_start(
                        xt[:, kb4 * 4:(kb4 + 1) * 4, :],
                        xt_d[:, kb4 * 4:(kb4 + 1) * 4, off:off + SC])

            def xt_slice(b, sc, kb, cols=None):
                xt = xt_t[(b, sc)]
                if isinstance(xt, tuple):
                    q = xt[1][kb // 4]
                    s = q[:, kb % 4, :]
                else:
                    s = xt[:, kb, :]
                return s if cols is None else s[:, cols]

            def emit_qk_pass(b, sc, pi, h):
                qt, kt, vt = ensure_qkv(b)
                w_sb, dstT, gam, pname = (
                    (wq_sb, qt, gq, "q") if pi == 0 else (wk_sb, kt, gk, "k"))
                ph = ps.tile([128, SC], F32, tag="ph", bufs=2,
                             name=f"ph_{pname}{b}_{sc}_{h}")
                for kb in range(NKB):
                    nc.tensor.matmul(
                        ph[:], w_sb[:, kb, h * 128:(h + 1) * 128],
                        xt_slice(b, sc, kb),
                        start=(kb == 0), stop=(kb == NKB - 1))
                sq = sqp.tile([128, SC], F16, tag="sq",
                              name=f"sq_{pname}{b}_{sc}_{h}")
                nc.scalar.activation(sq[:], ph[:], AF.Square)
                ssum = ps.tile([128, SC], F32, tag="sc", bufs=2,
                               name=f"ssum_{pname}{b}_{sc}_{h}")
                nc.tensor.matmul(ssum[:], ones16[:], sq[:],
                                 start=True, stop=True)
                # rstd = (mean(x^2)+eps)^-0.5 = exp(-0.5*ln(.))
                lg = tmp.tile([128, SC], F32, tag="lg",
                              name=f"lg_{pname}{b}_{sc}_{h}")
                nc.scalar.activation(lg[:], ssum[:], AF.Ln,
                                     bias=eps_sb[:], scale=1.0 / HEAD_DIM)
                rstd = tmp.tile([128, SC], F32, tag="rstd",
                                name=f"rstd_{pname}{b}_{sc}_{h}")
                nc.scalar.activation(rstd[:], lg[:], AF.Exp, scale=-0.5)
                nc.vector.scalar_tensor_tensor(
                    out=dstT[:, h, sc * SC:(sc + 1) * SC],
                    in0=ph[:], scalar=gam[:], in1=rstd[:],
                    op0=MUL, op1=MUL)

            def emit_v_pass(b, sc, j):
                _, _, vt = ensure_qkv(b)
                pv = ps.tile([128, IL], F32, tag="po", bufs=2,
                             name=f"pv{b}_{sc}_{j}")
                for kb in range(NKB):
                    nc.tensor.matmul(
                        pv[:], xt_slice(b, sc, kb,
                                        slice(j * 128, (j + 1) * 128)),
                        wv_sb[:, kb, :],
                        start=(kb == 0), stop=(kb == NKB - 1))
                nc.vector.tensor_copy(vt[:, sc * (SC // 128) + j, :], pv[:])

            def emit_wkwv_dma():
                for kb4 in range(NKB // 4):
                    nc.sync.dma_start(wk_sb[:, kb4 * 4:(kb4 + 1) * 4, :],
                                      Wk[:, kb4 * 4:(kb4 + 1) * 4, :])
                for kb4 in range(NKB // 4):
                    nc.sync.dma_start(wv_sb[:, kb4 * 4:(kb4 + 1) * 4, :],
                                      Wv[:, kb4 * 4:(kb4 + 1) * 4, :])

            def make_proj_units(b, first=False, defer_last=False):
                """Ordered emission units for batch b's projection phase.
                DMA for chunk sc+2 is emitted only after every chunk-sc
                unit (the xt ring-2 overwrite dependency tracks only
                already-emitted readers).  With defer_last, the final
                chunk's q and v passes are returned separately: they can
                run during batch b's OWN attention (vt[12..15] is first
                read ~7us into slot 0, qt cols 1536+ only at qb3), which
                moves PE work from the PE-bound merged phase into the
                ACT-bound final attention phase."""
                units = []
                units.append(lambda b=b: emit_xt_dma(b, 0, split_rings=first))
                if first:
                    units.append(emit_wkwv_dma)
                units.append(lambda b=b, f=first: emit_xt_dma(b, 1, rings2=f))
                late_v = []
                for sc in range(NSC):
                    defer = defer_last and sc == NSC - 1
                    for pi in range(2):
                        for h in range(HL):
                            units.append(
                                lambda b=b, sc=sc, pi=pi, h=h:
                                emit_qk_pass(b, sc, pi, h))
                    for j in range(SC // 128):
                        u = (lambda b=b, sc=sc, j=j: emit_v_pass(b, sc, j))
                        (late_v if defer else units).append(u)
                    if sc + 2 < NSC:
                        units.append(
                            lambda b=b, sc=sc, f=first:
                            emit_xt_dma(b, sc + 2, rings2=f))
                # vt[12..15] is read from kj 12 of the very first
                # attention slot, so the deferred v passes pump early.
                return units, late_v

            # ---- attention software-pipeline state ----
            pend_tail = [None]
            pend_po = [None]
            cur_attn_b = [0]
            po_units = []
            proj_units = []
            late_units = []

            def tail_step(step):
                """One stage of the previous slot's softmax tail.
                step 1: den matmul; 2: Ln; 3: recip exp; 4: ot mul."""
                if pend_tail[0] is None:
                    return
                av, roots, ot_t, h, bb, qq, st = pend_tail[0]
                if step == 1:
                    den = ps.tile([128, 512], F32, tag="po", bufs=2,
                                  name=f"den{bb}_{qq}_{h}")
                    nr = len(roots)
                    for i, rt in enumerate(roots):
                        nc.tensor.matmul(den[:], ones16[:], rt[:],
                                         start=(i == 0), stop=(i == nr - 1))
                    st["den"] = den
                elif step == 2:
                    lden = tmp.tile([128, 512], F32, tag="lden",
                                    name=f"lden{bb}_{qq}_{h}")
                    nc.scalar.activation(lden[:], st["den"][:], AF.Ln)
                    st["lden"] = lden
                elif step == 3:
                    r = tmp.tile([128, 512], F32, tag="r",
                                 name=f"r{bb}_{qq}_{h}")
                    nc.scalar.activation(r[:], st["lden"][:], AF.Exp,
                                         scale=-1.0)
                    st["r"] = r
                elif step == 4:
                    nc.vector.tensor_mul(ot_t[:, h, :], av[:], st["r"][:])
                    pend_tail[0] = None

            def po_load_units():
                if pend_po[0] is None:
                    return
                bb, qq, ot_t = pend_po[0]
                pend_po[0] = None
                for qs in range(4):
                    for dc in range(D // 512):
                        po_units.append((bb, qq, ot_t, qs, dc))

            def po_step(n=1, split_rings=False):
                """Emit n output-projection units (2 matmuls + copy + DMA).
                split_rings alternates the out-DMA between the sync and
                scalar HWDGE rings -- only safe when the scalar (ACT)
                queue has no pending exp work, i.e. at the final flush."""
                for i_ in range(n):
                    if not po_units:
                        return
                    bb, qq, ot_t, qs, dc = po_units.pop(0)
                    qi = qq * 4 + qs
                    po = ps.tile([128, 512], F32, tag="po", bufs=2,
                                 name=f"po{bb}_{qi}_{dc}")
                    for h in range(HL):
                        nc.tensor.matmul(
                            po[:], ot_t[:, h, qs * 128:(qs + 1) * 128],
                            wo_sb[:, h, dc * 512:(dc + 1) * 512],
                            start=(h == 0), stop=(h == HL - 1))
                    oo = oop.tile([128, 512], F16, tag="oo",
                                  name=f"oo{bb}_{qi}_{dc}")
                    nc.vector.tensor_copy(oo[:], po[:])
                    eng = nc.scalar if (split_rings and i_ % 2 == 1) else nc.sync
                    eng.dma_start(
                        out[bb * S + qi * 128: bb * S + (qi + 1) * 128,
                            dc * 512:(dc + 1) * 512], oo[:])

            def proj_step(n=1):
                for _ in range(n):
                    if not proj_units:
                        return
                    proj_units.pop(0)()

            def late_step(n=1):
                for _ in range(n):
                    if not late_units:
                        return
                    late_units.pop(0)()

            # ================= main schedule =================
            # batch-0 projection runs pure (nothing to overlap with).
            u0, _ = make_proj_units(0, first=True)
            for u in u0:
                u()
            nc.sync.dma_start(wo_sb[:], Wo[:])
            u1, l1 = make_proj_units(1, defer_last=True)
            proj_units.extend(u1)
            late_units.extend(l1)

            for b in range(B):
                cur_attn_b[0] = b
                if b > 0:
                    # safety: attention for b reads qt/kt/vt(b) -- every
                    # proj unit must be emitted by now (normally the
                    # pump points have already drained the list).
                    proj_step(len(proj_units))
                qt, kt, vt = ensure_qkv(b)
                for qb in range(NQB):
                    ot_qb = otp.tile([128, HL, 512], F16, tag="ot",
                                     name=f"ot{b}_{qb}")
                    for h in range(HL):
                        if h == HL - 1:
                            po_load_units()
                        av = ps.tile([128, 512], F32, tag="av", bufs=2,
                                     name=f"av{b}_{h}_{qb}")
                        roots = []
                        lvl_pend = [[] for _ in range(TREE_LVLS)]
                        exs = []

                        def av_and_tree(j, h=h, av=av, exs=exs,
                                        lvl_pend=lvl_pend, roots=roots,
                                        b=b, qb=qb):
                            # av matmul + pair-sum tree for key block j;
                            # runs one kj behind the score matmuls so the
                            # in-order PE never stalls on exp(j): the
                            # sc->exp->av latency chain paced the pure
                            # attention phase at ~19us/slot.
                            nc.tensor.matmul(
                                av[:], vt[:, j, h * 128:(h + 1) * 128],
                                exs[j][:],
                                start=(j == 0), stop=(j == NKJ - 1))
                            node = exs[j]
                            for lv in range(TREE_LVLS):
                                lvl_pend[lv].append(node)
                                if len(lvl_pend[lv]) < 2:
                                    node = None
                                    break
                                a0, a1 = lvl_pend[lv]
                                lvl_pend[lv] = []
                                t = trp.tile([128, 512], F16, tag=f"tr{lv}",
                                             bufs=(4 if lv < 3 else 3),
                                             name=f"tr{lv}_{b}_{h}_{qb}_{j}")
                                nc.vector.tensor_add(t[:], a0[:], a1[:])
                                node = t
                            if node is not None:
                                roots.append(node)

                        next_av = [0]
                        for kj in range(NKJ):
                            # once the proj-unit list is empty the ph ring
                            # has no future users: alternate the score
                            # psum between sc and ph tags (effective ring
                            # 4) and deepen the lookahead -- this covers
                            # the final batch AND the tail of the merged
                            # phase, whose proj units run out early.
                            free_ph = (b == B - 1) or not proj_units
                            stag = "ph" if (free_ph and kj % 2 == 1) \
                                else "sc"
                            sc_ps = ps.tile([128, 512], F32, tag=stag,
                                            bufs=2,
                                            name=f"sc{b}_{h}_{qb}_{kj}")
                            nc.tensor.matmul(
                                sc_ps[:], kt[:, h, kj * 128:(kj + 1) * 128],
                                qt[:, h, qb * 512:(qb + 1) * 512],
                                start=True, stop=True)
                            ex = expool.tile([128, 512], F16, tag="ex",
                                             name=f"ex{b}_{h}_{qb}_{kj}")
                            nc.scalar.activation(ex[:], sc_ps[:], AF.Exp,
                                                 scale=scale)
                            exs.append(ex)
                            # lookahead depth 2 when the ph alternation is
                            # active (effective sc ring 4), else 1.
                            lag = 2 if free_ph else 1
                            if kj == 0:
                                po_step(1)
                            while next_av[0] <= kj - lag:
                                av_and_tree(next_av[0])
                                next_av[0] += 1
                            # pump slack work into the PE's exp-wait gaps
                            # (the PE executes matmuls in program order).
                            if kj in (1, 2, 3, 4):
                                tail_step(kj)
                            if kj % 2 == 1:
                                po_step(1)
                            if kj in PROJ_PUMP_KJ:
                                proj_step(1)
                            if b == B - 1 and kj in (2, 4, 6, 8):
                                # deferred b1 v passes: vt[12..15] is
                                # first read at kj 12 of this slot.
                                late_step(1)
                        while next_av[0] < NKJ:
                            av_and_tree(next_av[0])
                            next_av[0] += 1
                        pend_tail[0] = (av, roots, ot_qb, h, b, qb, {})
                    # drain toward 2 (keep kj0 gap-fillers) but cap the
                    # burst so the ACT exp pipeline never starves behind
                    # a long run of po matmuls.
                    po_step(min(max(0, len(po_units) - 2), 8))
                    pend_po[0] = (b, qb, ot_qb)

            # final flush
            proj_step(len(proj_units))
            late_step(len(late_units))
            for stp in (1, 2, 3, 4):
                tail_step(stp)
            po_load_units()
            po_step(len(po_units), split_rings=True)

    if split:
        _split_waits(nc)
    return nc


def _prep_in_maps(inputs, B, S, D, HL):
    """Shard + pre-tile the full inputs for the 8 cores (all fp16)."""
    BS = B * S
    NKB = D // 128
    IL = HL * HEAD_DIM
    x = np.asarray(inputs["x"], dtype=np.float32)
    Wq = np.asarray(inputs["Wq"], dtype=np.float32)
    Wk = np.asarray(inputs["Wk"], dtype=np.float32)
    Wv = np.asarray(inputs["Wv"], dtype=np.float32)
    Wo = np.asarray(inputs["Wo"], dtype=np.float32)
    qg = np.ascontiguousarray(
        np.asarray(inputs["q_gamma"], dtype=np.float32).reshape(128, 1))
    kg = np.ascontiguousarray(
        np.asarray(inputs["k_gamma"], dtype=np.float32).reshape(128, 1))
    # x[bs, kb*128+p] -> [p, kb, bs]
    xtld = np.ascontiguousarray(
        x.reshape(BS, NKB, 128).transpose(2, 1, 0).astype(np.float16))
    in_maps = []
    for c in range(N_CORES):
        cs = slice(c * IL, (c + 1) * IL)
        # W[kb*128+p, n] -> [p, kb, n]
        wq_t = np.ascontiguousarray(
            Wq[:, cs].reshape(NKB, 128, IL).transpose(1, 0, 2).astype(np.float16))
        wk_t = np.ascontiguousarray(
            Wk[:, cs].reshape(NKB, 128, IL).transpose(1, 0, 2).astype(np.float16))
        wv_t = np.ascontiguousarray(
            Wv[:, cs].reshape(NKB, 128, IL).transpose(1, 0, 2).astype(np.float16))
        # Wo[h*128+p, :] -> [p, h, :]
        wo_t = np.ascontiguousarray(
            Wo[cs, :].reshape(HL, 128, D).transpose(1, 0, 2).astype(np.float16))
        in_maps.append({
            "xt_d": xtld,
            "Wq": wq_t,
            "Wk": wk_t,
            "Wv": wv_t,
            "Wo": wo_t,
            "qg": qg,
            "kg": kg,
        })
    return in_maps


_NC_CACHE = {}


def run_cores(inputs, trace=False):
    """Build (cached), shard, run on 8 cores; returns (full_out, results)."""
    from concourse.bass_utils import run_bass_kernel_spmd

    x = np.asarray(inputs["x"])
    B, S, D = x.shape
    HL = N_HEADS // N_CORES
    key = (B, S, D, HL)
    if key not in _NC_CACHE:
        _NC_CACHE[key] = build_nc(B, S, D, HL)
    nc = _NC_CACHE[key]
    in_maps = _prep_in_maps(inputs, B, S, D, HL)
    res = run_bass_kernel_spmd(nc, in_maps, list(range(N_CORES)), trace=trace)
    acc = res.results[0]["out"].astype(np.float32)
    for c in range(1, N_CORES):
        acc = acc + res.results[c]["out"].astype(np.float32)
    return acc.reshape(B, S, D), res


def kernel(**inputs) -> np.ndarray:
    return run_cores(inputs, trace=False)[0]


# revision 27
# speedup vs baseline: 1.0115x; 1.0032x over previous
"""Tensor-parallel multi-head attention kernel for 8 Trainium2 NeuronCores.

Problem: nn_Attention (B=2, S=2048, D=2048, 16 heads x 128) with per-head
RMSNorm on q/k, non-causal softmax attention, and output projection.

Sharding (tensor-parallel over heads, per the hint):
  - core c owns heads {2c, 2c+1}: Wq/Wk/Wv column slices [D, 256], Wo row
    slice [256, D].
  - every core reads all of x (the projection contracts over full D and
    full sequence is needed for non-causal attention keys/values).
  - each core emits a partial output  out_c = attn_out_c @ Wo_c ; the
    host unshard sums the 8 partials (the natural unshard for row-sharded
    Wo -- equivalent to the all-reduce in the hint, done at gather time).

Pipeline structure (v3):
  - fp16 everywhere device-side; host pre-tiles inputs and sums fp16
    partials in f32.  Matmul rate is unchanged vs f32r but LDWEIGHTS
    runs 2+ elem/cycle (FWL), DMA bytes halve, DVE gets its 2x mode.
  - softmax denominator via a DVE pair-sum tree (fp16 2x) + a single
    accumulating ones-matmul; the old 16 ones-matmuls were 60us of PE.
  - the attention kj loop is ACT-exp paced (~610ns/exp vs 466ns of PE
    work) and the PE executes matmuls in program order, so all slack
    work is PUMPED into the kj loop: the previous slot's den/recip
    chain (kj 1-4), the previous query block's output projection (odd
    kj), and -- during batch-0 attention -- batch-1's entire
    projection+rmsnorm phase, one pass per pump point (kj 5,7,9,11,13).
    This keeps the PE saturated through the ACT-bound attention phase
    and removes the batch boundary entirely (qkv tiles double-buffer).
  - PSUM banks (8): attn scores + proj ssum ring 2, proj q/k ring 2,
    attn av ring 2, {outproj, v-proj, den} ring 2.
  - startup: first matmul depends only on the first wq + xt chunks;
    DMA order is wq, xt(chunk0), wk, wv, xt(chunk1), ... so the PE
    starts as soon as ~1MB has landed instead of after all weights.
"""

import math
import sys

for _p in ("/opt/trn_rl_repo",):
    if _p not in sys.path:
        sys.path.insert(0, _p)

import numpy as np

import bass_rust
import concourse.bass as bass
import concourse.mybir as mybir
import concourse.tile as tile

F32 = mybir.dt.float32
F16 = mybir.dt.float16
AF = mybir.ActivationFunctionType
MUL = mybir.AluOpType.mult

N_CORES = 8
N_HEADS = 16
HEAD_DIM = 128
EPS = 1e-6
TREE_LVLS = 4      # levels of DVE pair-summing before the ones-matmul
PROJ_PUMP_KJ = (5, 7, 9, 11, 13)   # proj-unit pump points in the kj loop

_wait_counter = [0]


def _split_waits(nc, limit=1):
    """This compiler build rejects >1 semaphore wait per instruction
    ("Too many sync wait commands").  Move excess waits onto preceding
    same-engine no-ops: the sequencer executes them in order, so waiting
    earlier on the same engine is semantically equivalent."""
    for fn in nc.m.functions:
        for blk in fn.blocks:
            newl = []
            changed = False
            for inst in blk.instructions:
                si = inst.sync_info
                waits = list(si.on_wait) if si is not None and si.on_wait else []
                if len(waits) > limit:
                    extra, keep = waits[:-limit], waits[-limit:]
                    for w in extra:
                        _wait_counter[0] += 1
                        nop = bass_rust.InstNoOp(name=f"I-waitsplit-{_wait_counter[0]}")
                        nop.engine = inst.engine
                        nop.sync_info = mybir.SyncInfo(on_wait=[w], on_update=[])
                        newl.append(nop)
                    si.on_wait = keep
                    changed = True
                newl.append(inst)
            if changed:
                blk.instructions = newl


def build_nc(B, S, D, HL, split=True):
    """Emit the per-core program. HL = heads per core."""
    IL = HL * HEAD_DIM          # local inner dim
    NKB = D // 128              # contraction blocks for projections
    SC = 512                    # seq chunk for the projection phase
    NSC = S // SC
    NQB = S // 512              # query blocks in attention
    NKJ = S // 128              # key blocks in attention
    BS = B * S
    scale = 1.0 / math.sqrt(HEAD_DIM)

    nc = bass.Bass("TRN2", target_bir_lowering=False, debug=False,
                   num_devices=N_CORES)
    # host-pre-tiled layouts: [partition, kb, free]
    xt_d = nc.dram_tensor("xt_d", [128, NKB, BS], F16, kind="ExternalInput")
    Wq = nc.dram_tensor("Wq", [128, NKB, IL], F16, kind="ExternalInput")
    Wk = nc.dram_tensor("Wk", [128, NKB, IL], F16, kind="ExternalInput")
    Wv = nc.dram_tensor("Wv", [128, NKB, IL], F16, kind="ExternalInput")
    Wo = nc.dram_tensor("Wo", [128, HL, D], F16, kind="ExternalInput")
    qg = nc.dram_tensor("qg", [128, 1], F32, kind="ExternalInput")
    kg = nc.dram_tensor("kg", [128, 1], F32, kind="ExternalInput")
    out = nc.dram_tensor("out", [BS, D], F16, kind="ExternalOutput")

    with tile.TileContext(nc) as tc:
        with (
            tc.tile_pool(name="wpool", bufs=1) as wpool,
            tc.tile_pool(name="xpool", bufs=2) as xpool,
            tc.tile_pool(name="x0q", bufs=1) as x0q,
            tc.tile_pool(name="qkv", bufs=2) as qkv,
            tc.tile_pool(name="otp", bufs=5) as otp,
            tc.tile_pool(name="tmp", bufs=2) as tmp,
            tc.tile_pool(name="sqp", bufs=4) as sqp,
            tc.tile_pool(name="expool", bufs=8) as expool,
            tc.tile_pool(name="trp", bufs=4) as trp,
            tc.tile_pool(name="oop", bufs=10) as oop,
            tc.tile_pool(name="ps", bufs=1, space="PSUM") as ps,
        ):
            # ---- resident constants / weights ----
            wq_sb = wpool.tile([128, NKB, IL], F16, name="wq_sb")
            for kb4 in range(NKB // 4):
                nc.sync.dma_start(wq_sb[:, kb4 * 4:(kb4 + 1) * 4, :],
                                  Wq[:, kb4 * 4:(kb4 + 1) * 4, :])
            gq = wpool.tile([128, 1], F32, name="gq")
            nc.sync.dma_start(gq[:], qg[:])
            gk = wpool.tile([128, 1], F32, name="gk")
            nc.sync.dma_start(gk[:], kg[:])
            wk_sb = wpool.tile([128, NKB, IL], F16, name="wk_sb")
            wv_sb = wpool.tile([128, NKB, IL], F16, name="wv_sb")
            wo_sb = wpool.tile([128, HL, D], F16, name="wo_sb")
            ones_f32 = wpool.tile([128, 128], F32, name="ones_f32")
            nc.vector.memset(ones_f32[:], 1.0)
            ones16 = wpool.tile([128, 128], F16, name="ones16")
            nc.scalar.copy(ones16[:], ones_f32[:])
            eps_sb = wpool.tile([128, 1], F32, name="eps_sb")
            nc.vector.memset(eps_sb[:], EPS)
            # HAM warmup: the PE cold-throttles (K=4/8, ~2x slow) for the
            # first ~4us of matmuls.  Run dummy ones-matmuls while the
            # startup DMAs are in flight so real matmuls start at full
            # rate.  No readers -> the po-ring slots recycle instantly.
            for wu in range(60):
                wu_ps = ps.tile([128, 128], F32, tag="po", bufs=2,
                                name=f"wu{wu}")
                nc.tensor.matmul(wu_ps[:], ones16[:], ones16[:],
                                 start=True, stop=True)

            # ================= emission helpers =================
            # qkv tiles per batch (ring 2 => batches coexist)
            qkv_t = {}

            def ensure_qkv(b):
                if b not in qkv_t:
                    qkv_t[b] = (
                        qkv.tile([128, HL, S], F16, tag="qt", name=f"qt{b}"),
                        qkv.tile([128, HL, S], F16, tag="kt", name=f"kt{b}"),
                        qkv.tile([128, NKJ, IL], F16, tag="vt", name=f"vt{b}"),
                    )
                return qkv_t[b]

            xt_t = {}

            def emit_xt_dma(b, sc, split_rings=False, rings2=False):
                off = b * S + sc * SC
                if split_rings:
                    # startup chunk: four independent quarter tiles on the
                    # scalar HWDGE ring (weights stream on the sync ring in
                    # parallel), so the first matmul waits only for the
                    # first 512KB quarter instead of the whole 2MB chunk.
                    quads = []
                    for kb4 in range(NKB // 4):
                        q = x0q.tile([128, 4, SC], F16, tag=f"x0q{kb4}",
                                     name=f"x0q_{kb4}")
                        nc.scalar.dma_start(
                            q[:], xt_d[:, kb4 * 4:(kb4 + 1) * 4,
                                       off:off + SC])
                        quads.append(q)
                    xt_t[(b, sc)] = ("quad", quads)
                    return
                xt = xpool.tile([128, NKB, SC], F16, tag="xt",
                                name=f"xt{b}_{sc}")
                xt_t[(b, sc)] = xt
                for kb4 in range(NKB // 4):
                    # batch-0 chunks stream on both HWDGE rings (the ACT
                    # queue is ~40% busy in pure proj); batch-1 chunks
                    # stay on sync (ACT paces the merged attention).
                    eng = nc.scalar if (rings2 and kb4 < 2) else nc.sync
                    eng.dma_start(
                        xt[:, kb4 * 4:(kb4 + 1) * 4, :],
                        xt_d[:, kb4 * 4:(kb4 + 1) * 4, off:off + SC])

            def xt_slice(b, sc, kb, cols=None):
                xt = xt_t[(b, sc)]
                if isinstance(xt, tuple):
                    q = xt[1][kb // 4]
                    s = q[:, kb % 4, :]
                else:
                    s = xt[:, kb, :]
                return s if cols is None else s[:, cols]

            def emit_qk_pass(b, sc, pi, h):
                qt, kt, vt = ensure_qkv(b)
                w_sb, dstT, gam, pname = (
                    (wq_sb, qt, gq, "q") if pi == 0 else (wk_sb, kt, gk, "k"))
                ph = ps.tile([128, SC], F32, tag="ph", bufs=2,
                             name=f"ph_{pname}{b}_{sc}_{h}")
                for kb in range(NKB):
                    nc.tensor.matmul(
                        ph[:], w_sb[:, kb, h * 128:(h + 1) * 128],
                        xt_slice(b, sc, kb),
                        start=(kb == 0), stop=(kb == NKB - 1))
                sq = sqp.tile([128, SC], F16, tag="sq",
                              name=f"sq_{pname}{b}_{sc}_{h}")
                nc.scalar.activation(sq[:], ph[:], AF.Square)
                ssum = ps.tile([128, SC], F32, tag="sc", bufs=2,
                               name=f"ssum_{pname}{b}_{sc}_{h}")
                nc.tensor.matmul(ssum[:], ones16[:], sq[:],
                                 start=True, stop=True)
                # rstd = (mean(x^2)+eps)^-0.5 = exp(-0.5*ln(.))
                lg = tmp.tile([128, SC], F32, tag="lg",
                              name=f"lg_{pname}{b}_{sc}_{h}")
                nc.scalar.activation(lg[:], ssum[:], AF.Ln,
                                     bias=eps_sb[:], scale=1.0 / HEAD_DIM)
                rstd = tmp.tile([128, SC], F32, tag="rstd",
                                name=f"rstd_{pname}{b}_{sc}_{h}")
                nc.scalar.activation(rstd[:], lg[:], AF.Exp, scale=-0.5)
                nc.vector.scalar_tensor_tensor(
                    out=dstT[:, h, sc * SC:(sc + 1) * SC],
                    in0=ph[:], scalar=gam[:], in1=rstd[:],
                    op0=MUL, op1=MUL)

            def emit_v_pass(b, sc, j):
                _, _, vt = ensure_qkv(b)
                pv = ps.tile([128, IL], F32, tag="po", bufs=2,
                             name=f"pv{b}_{sc}_{j}")
                for kb in range(NKB):
                    nc.tensor.matmul(
                        pv[:], xt_slice(b, sc, kb,
                                        slice(j * 128, (j + 1) * 128)),
                        wv_sb[:, kb, :],
                        start=(kb == 0), stop=(kb == NKB - 1))
                nc.vector.tensor_copy(vt[:, sc * (SC // 128) + j, :], pv[:])

            def emit_wkwv_dma():
                for kb4 in range(NKB // 4):
                    nc.sync.dma_start(wk_sb[:, kb4 * 4:(kb4 + 1) * 4, :],
                                      Wk[:, kb4 * 4:(kb4 + 1) * 4, :])
                for kb4 in range(NKB // 4):
                    nc.sync.dma_start(wv_sb[:, kb4 * 4:(kb4 + 1) * 4, :],
                                      Wv[:, kb4 * 4:(kb4 + 1) * 4, :])

            def make_proj_units(b, first=False, defer_last=False):
                """Ordered emission units for batch b's projection phase.
                DMA for chunk sc+2 is emitted only after every chunk-sc
                unit (the xt ring-2 overwrite dependency tracks only
                already-emitted readers).  With defer_last, the final
                chunk's q and v passes are returned separately: they can
                run during batch b's OWN attention (vt[12..15] is first
                read ~7us into slot 0, qt cols 1536+ only at qb3), which
                moves PE work from the PE-bound merged phase into the
                ACT-bound final attention phase."""
                units = []
                units.append(lambda b=b: emit_xt_dma(b, 0, split_rings=first))
                if first:
                    units.append(emit_wkwv_dma)
                units.append(lambda b=b, f=first: emit_xt_dma(b, 1, rings2=f))
                late_v = []
                for sc in range(NSC):
                    defer = defer_last and sc == NSC - 1
                    for pi in range(2):
                        for h in range(HL):
                            units.append(
                                lambda b=b, sc=sc, pi=pi, h=h:
                                emit_qk_pass(b, sc, pi, h))
                    for j in range(SC // 128):
                        u = (lambda b=b, sc=sc, j=j: emit_v_pass(b, sc, j))
                        (late_v if defer else units).append(u)
                    if sc + 2 < NSC:
                        units.append(
                            lambda b=b, sc=sc, f=first:
                            emit_xt_dma(b, sc + 2, rings2=f))
                # vt[12..15] is read from kj 12 of the very first
                # attention slot, so the deferred v passes pump early.
                return units, late_v

            # ---- attention software-pipeline state ----
            pend_tail = [None]
            pend_po = [None]
            cur_attn_b = [0]
            po_units = []
            proj_units = []
            late_units = []

            def tail_step(step):
                """One stage of the previous slot's softmax tail.
                step 1: den matmul; 2: Ln; 3: recip exp; 4: ot mul."""
                if pend_tail[0] is None:
                    return
                av, roots, ot_t, h, bb, qq, st = pend_tail[0]
                if step == 1:
                    den = ps.tile([128, 512], F32, tag="po", bufs=2,
                                  name=f"den{bb}_{qq}_{h}")
                    nr = len(roots)
                    for i, rt in enumerate(roots):
                        nc.tensor.matmul(den[:], ones16[:], rt[:],
                                         start=(i == 0), stop=(i == nr - 1))
                    st["den"] = den
                elif step == 2:
                    lden = tmp.tile([128, 512], F32, tag="lden",
                                    name=f"lden{bb}_{qq}_{h}")
                    nc.scalar.activation(lden[:], st["den"][:], AF.Ln)
                    st["lden"] = lden
                elif step == 3:
                    r = tmp.tile([128, 512], F32, tag="r",
                                 name=f"r{bb}_{qq}_{h}")
                    nc.scalar.activation(r[:], st["lden"][:], AF.Exp,
                                         scale=-1.0)
                    st["r"] = r
                elif step == 4:
                    nc.vector.tensor_mul(ot_t[:, h, :], av[:], st["r"][:])
                    pend_tail[0] = None

            def po_load_units():
                if pend_po[0] is None:
                    return
                bb, qq, ot_t = pend_po[0]
                pend_po[0] = None
                for qs in range(4):
                    for dc in range(D // 512):
                        po_units.append((bb, qq, ot_t, qs, dc))

            def po_step(n=1, split_rings=False):
                """Emit n output-projection units (2 matmuls + copy + DMA).
                split_rings alternates the out-DMA between the sync and
                scalar HWDGE rings -- only safe when the scalar (ACT)
                queue has no pending exp work, i.e. at the final flush."""
                for i_ in range(n):
                    if not po_units:
                        return
                    bb, qq, ot_t, qs, dc = po_units.pop(0)
                    qi = qq * 4 + qs
                    po = ps.tile([128, 512], F32, tag="po", bufs=2,
                                 name=f"po{bb}_{qi}_{dc}")
                    for h in range(HL):
                        nc.tensor.matmul(
                            po[:], ot_t[:, h, qs * 128:(qs + 1) * 128],
                            wo_sb[:, h, dc * 512:(dc + 1) * 512],
                            start=(h == 0), stop=(h == HL - 1))
                    oo = oop.tile([128, 512], F16, tag="oo",
                                  name=f"oo{bb}_{qi}_{dc}")
                    if split_rings and i_ % 2 == 1:
                        # at the final flush the exp stream is done, so
                        # ACT can absorb half the psum->fp16 copies that
                        # otherwise serialize on the DVE in the tail.
                        nc.scalar.copy(oo[:], po[:])
                    else:
                        nc.vector.tensor_copy(oo[:], po[:])
                    eng = nc.scalar if (split_rings and i_ % 2 == 1) else nc.sync
                    eng.dma_start(
                        out[bb * S + qi * 128: bb * S + (qi + 1) * 128,
                            dc * 512:(dc + 1) * 512], oo[:])

            def proj_step(n=1):
                for _ in range(n):
                    if not proj_units:
                        return
                    proj_units.pop(0)()

            def late_step(n=1):
                for _ in range(n):
                    if not late_units:
                        return
                    late_units.pop(0)()

            # ================= main schedule =================
            # batch-0 projection runs pure (nothing to overlap with).
            u0, _ = make_proj_units(0, first=True)
            for u in u0:
                u()
            nc.sync.dma_start(wo_sb[:], Wo[:])
            u1, l1 = make_proj_units(1, defer_last=True)
            proj_units.extend(u1)
            late_units.extend(l1)

            for b in range(B):
                cur_attn_b[0] = b
                if b > 0:
                    # safety: attention for b reads qt/kt/vt(b) -- every
                    # proj unit must be emitted by now (normally the
                    # pump points have already drained the list).
                    proj_step(len(proj_units))
                qt, kt, vt = ensure_qkv(b)
                for qb in range(NQB):
                    ot_qb = otp.tile([128, HL, 512], F16, tag="ot",
                                     name=f"ot{b}_{qb}")
                    for h in range(HL):
                        if h == HL - 1:
                            po_load_units()
                        av = ps.tile([128, 512], F32, tag="av", bufs=2,
                                     name=f"av{b}_{h}_{qb}")
                        roots = []
                        lvl_pend = [[] for _ in range(TREE_LVLS)]
                        exs = []

                        def av_and_tree(j, h=h, av=av, exs=exs,
                                        lvl_pend=lvl_pend, roots=roots,
                                        b=b, qb=qb):
                            # av matmul + pair-sum tree for key block j;
                            # runs one kj behind the score matmuls so the
                            # in-order PE never stalls on exp(j): the
                            # sc->exp->av latency chain paced the pure
                            # attention phase at ~19us/slot.
                            nc.tensor.matmul(
                                av[:], vt[:, j, h * 128:(h + 1) * 128],
                                exs[j][:],
                                start=(j == 0), stop=(j == NKJ - 1))
                            node = exs[j]
                            for lv in range(TREE_LVLS):
                                lvl_pend[lv].append(node)
                                if len(lvl_pend[lv]) < 2:
                                    node = None
                                    break
                                a0, a1 = lvl_pend[lv]
                                lvl_pend[lv] = []
                                t = trp.tile([128, 512], F16, tag=f"tr{lv}",
                                             bufs=(4 if lv < 3 else 3),
                                             name=f"tr{lv}_{b}_{h}_{qb}_{j}")
                                nc.vector.tensor_add(t[:], a0[:], a1[:])
                                node = t
                            if node is not None:
                                roots.append(node)

                        next_av = [0]
                        for kj in range(NKJ):
                            # once the proj-unit list is empty the ph ring
                            # has no future users: alternate the score
                            # psum between sc and ph tags (effective ring
                            # 4) and deepen the lookahead -- this covers
                            # the final batch AND the tail of the merged
                            # phase, whose proj units run out early.
                            free_ph = (b == B - 1) or not proj_units
                            stag = "ph" if (free_ph and kj % 2 == 1) \
                                else "sc"
                            sc_ps = ps.tile([128, 512], F32, tag=stag,
                                            bufs=2,
                                            name=f"sc{b}_{h}_{qb}_{kj}")
                            nc.tensor.matmul(
                                sc_ps[:], kt[:, h, kj * 128:(kj + 1) * 128],
                                qt[:, h, qb * 512:(qb + 1) * 512],
                                start=True, stop=True)
                            ex = expool.tile([128, 512], F16, tag="ex",
                                             name=f"ex{b}_{h}_{qb}_{kj}")
                            nc.scalar.activation(ex[:], sc_ps[:], AF.Exp,
                                                 scale=scale)
                            exs.append(ex)
                            # lookahead depth 2 when the ph alternation is
                            # active (effective sc ring 4), else 1.
                            lag = 2 if free_ph else 1
                            if kj == 0:
                                po_step(1)
                            while next_av[0] <= kj - lag:
                                av_and_tree(next_av[0])
                                next_av[0] += 1
                            # pump slack work into the PE's exp-wait gaps
                            # (the PE executes matmuls in program order).
                            if kj in (1, 2, 3, 4):
                                tail_step(kj)
                            if kj % 2 == 1:
                                po_step(1)
                            if kj in PROJ_PUMP_KJ:
                                proj_step(1)
                            if b == B - 1 and kj in (2, 4, 6, 8):
                                # deferred b1 v passes: vt[12..15] is
                                # first read at kj 12 of this slot.
                                late_step(1)
                        while next_av[0] < NKJ:
                            av_and_tree(next_av[0])
                            next_av[0] += 1
                        pend_tail[0] = (av, roots, ot_qb, h, b, qb, {})
                    # drain toward 2 (keep kj0 gap-fillers) but cap the
                    # burst so the ACT exp pipeline never starves behind
                    # a long run of po matmuls.
                    po_step(min(max(0, len(po_units) - 2), 8))
                    pend_po[0] = (b, qb, ot_qb)

            # final flush
            proj_step(len(proj_units))
            late_step(len(late_units))
            for stp in (1, 2, 3, 4):
                tail_step(stp)
            po_load_units()
            po_step(len(po_units), split_rings=True)

    if split:
        _split_waits(nc)
    return nc


def _prep_in_maps(inputs, B, S, D, HL):
    """Shard + pre-tile the full inputs for the 8 cores (all fp16)."""
    BS = B * S
    NKB = D // 128
    IL = HL * HEAD_DIM
    x = np.asarray(inputs["x"], dtype=np.float32)
    Wq = np.asarray(inputs["Wq"], dtype=np.float32)
    Wk = np.asarray(inputs["Wk"], dtype=np.float32)
    Wv = np.asarray(inputs["Wv"], dtype=np.float32)
    Wo = np.asarray(inputs["Wo"], dtype=np.float32)
    qg = np.ascontiguousarray(
        np.asarray(inputs["q_gamma"], dtype=np.float32).reshape(128, 1))
    kg = np.ascontiguousarray(
        np.asarray(inputs["k_gamma"], dtype=np.float32).reshape(128, 1))
    # x[bs, kb*128+p] -> [p, kb, bs]
    xtld = np.ascontiguousarray(
        x.reshape(BS, NKB, 128).transpose(2, 1, 0).astype(np.float16))
    in_maps = []
    for c in range(N_CORES):
        cs = slice(c * IL, (c + 1) * IL)
        # W[kb*128+p, n] -> [p, kb, n]
        wq_t = np.ascontiguousarray(
            Wq[:, cs].reshape(NKB, 128, IL).transpose(1, 0, 2).astype(np.float16))
        wk_t = np.ascontiguousarray(
            Wk[:, cs].reshape(NKB, 128, IL).transpose(1, 0, 2).astype(np.float16))
        wv_t = np.ascontiguousarray(
            Wv[:, cs].reshape(NKB, 128, IL).transpose(1, 0, 2).astype(np.float16))
        # Wo[h*128+p, :] -> [p, h, :]
        wo_t = np.ascontiguousarray(
            Wo[cs, :].reshape(HL, 128, D).transpose(1, 0, 2).astype(np.float16))
        in_maps.append({
            "xt_d": xtld,
            "Wq": wq_t,
            "Wk": wk_t,
            "Wv": wv_t,
            "Wo": wo_t,
            "qg": qg,
            "kg": kg,
        })
    return in_maps


_NC_CACHE = {}


def run_cores(inputs, trace=False):
    """Build (cached), shard, run on 8 cores; returns (full_out, results)."""
    from concourse.bass_utils import run_bass_kernel_spmd

    x = np.asarray(inputs["x"])
    B, S, D = x.shape
    HL = N_HEADS // N_CORES
    key = (B, S, D, HL)
    if key not in _NC_CACHE:
        _NC_CACHE[key] = build_nc(B, S, D, HL)
    nc = _NC_CACHE[key]
    in_maps = _prep_in_maps(inputs, B, S, D, HL)
    res = run_bass_kernel_spmd(nc, in_maps, list(range(N_CORES)), trace=trace)
    acc = res.results[0]["out"].astype(np.float32)
    for c in range(1, N_CORES):
        acc = acc + res.results[c]["out"].astype(np.float32)
    return acc.reshape(B, S, D), res


def kernel(**inputs) -> np.ndarray:
    return run_cores(inputs, trace=False)[0]


# revision 28
# speedup vs baseline: 1.0137x; 1.0022x over previous
"""Tensor-parallel multi-head attention kernel for 8 Trainium2 NeuronCores.

Problem: nn_Attention (B=2, S=2048, D=2048, 16 heads x 128) with per-head
RMSNorm on q/k, non-causal softmax attention, and output projection.

Sharding (tensor-parallel over heads, per the hint):
  - core c owns heads {2c, 2c+1}: Wq/Wk/Wv column slices [D, 256], Wo row
    slice [256, D].
  - every core reads all of x (the projection contracts over full D and
    full sequence is needed for non-causal attention keys/values).
  - each core emits a partial output  out_c = attn_out_c @ Wo_c ; the
    host unshard sums the 8 partials (the natural unshard for row-sharded
    Wo -- equivalent to the all-reduce in the hint, done at gather time).

Pipeline structure (v3):
  - fp16 everywhere device-side; host pre-tiles inputs and sums fp16
    partials in f32.  Matmul rate is unchanged vs f32r but LDWEIGHTS
    runs 2+ elem/cycle (FWL), DMA bytes halve, DVE gets its 2x mode.
  - softmax denominator via a DVE pair-sum tree (fp16 2x) + a single
    accumulating ones-matmul; the old 16 ones-matmuls were 60us of PE.
  - the attention kj loop is ACT-exp paced (~610ns/exp vs 466ns of PE
    work) and the PE executes matmuls in program order, so all slack
    work is PUMPED into the kj loop: the previous slot's den/recip
    chain (kj 1-4), the previous query block's output projection (odd
    kj), and -- during batch-0 attention -- batch-1's entire
    projection+rmsnorm phase, one pass per pump point (kj 5,7,9,11,13).
    This keeps the PE saturated through the ACT-bound attention phase
    and removes the batch boundary entirely (qkv tiles double-buffer).
  - PSUM banks (8): attn scores + proj ssum ring 2, proj q/k ring 2,
    attn av ring 2, {outproj, v-proj, den} ring 2.
  - startup: first matmul depends only on the first wq + xt chunks;
    DMA order is wq, xt(chunk0), wk, wv, xt(chunk1), ... so the PE
    starts as soon as ~1MB has landed instead of after all weights.
"""

import math
import sys

for _p in ("/opt/trn_rl_repo",):
    if _p not in sys.path:
        sys.path.insert(0, _p)

import numpy as np

import bass_rust
import concourse.bass as bass
import concourse.mybir as mybir
import concourse.tile as tile

F32 = mybir.dt.float32
F16 = mybir.dt.float16
AF = mybir.ActivationFunctionType
MUL = mybir.AluOpType.mult

N_CORES = 8
N_HEADS = 16
HEAD_DIM = 128
EPS = 1e-6
TREE_LVLS = 4      # levels of DVE pair-summing before the ones-matmul
PROJ_PUMP_KJ = (5, 7, 9, 11, 13)   # proj-unit pump points in the kj loop

_wait_counter = [0]


def _split_waits(nc, limit=1):
    """This compiler build rejects >1 semaphore wait per instruction
    ("Too many sync wait commands").  Move excess waits onto preceding
    same-engine no-ops: the sequencer executes them in order, so waiting
    earlier on the same engine is semantically equivalent."""
    for fn in nc.m.functions:
        for blk in fn.blocks:
            newl = []
            changed = False
            for inst in blk.instructions:
                si = inst.sync_info
                waits = list(si.on_wait) if si is not None and si.on_wait else []
                if len(waits) > limit:
                    extra, keep = waits[:-limit], waits[-limit:]
                    for w in extra:
                        _wait_counter[0] += 1
                        nop = bass_rust.InstNoOp(name=f"I-waitsplit-{_wait_counter[0]}")
                        nop.engine = inst.engine
                        nop.sync_info = mybir.SyncInfo(on_wait=[w], on_update=[])
                        newl.append(nop)
                    si.on_wait = keep
                    changed = True
                newl.append(inst)
            if changed:
                blk.instructions = newl


def build_nc(B, S, D, HL, split=True):
    """Emit the per-core program. HL = heads per core."""
    IL = HL * HEAD_DIM          # local inner dim
    NKB = D // 128              # contraction blocks for projections
    SC = 512                    # seq chunk for the projection phase
    NSC = S // SC
    NQB = S // 512              # query blocks in attention
    NKJ = S // 128              # key blocks in attention
    BS = B * S
    scale = 1.0 / math.sqrt(HEAD_DIM)

    nc = bass.Bass("TRN2", target_bir_lowering=False, debug=False,
                   num_devices=N_CORES)
    # host-pre-tiled layouts: [partition, kb, free]
    xt_d = nc.dram_tensor("xt_d", [128, NKB, BS], F16, kind="ExternalInput")
    Wq = nc.dram_tensor("Wq", [128, NKB, IL], F16, kind="ExternalInput")
    Wk = nc.dram_tensor("Wk", [128, NKB, IL], F16, kind="ExternalInput")
    Wv = nc.dram_tensor("Wv", [128, NKB, IL], F16, kind="ExternalInput")
    Wo = nc.dram_tensor("Wo", [128, HL, D], F16, kind="ExternalInput")
    qg = nc.dram_tensor("qg", [128, 1], F32, kind="ExternalInput")
    kg = nc.dram_tensor("kg", [128, 1], F32, kind="ExternalInput")
    out = nc.dram_tensor("out", [BS, D], F16, kind="ExternalOutput")

    with tile.TileContext(nc) as tc:
        with (
            tc.tile_pool(name="wpool", bufs=1) as wpool,
            tc.tile_pool(name="xpool", bufs=2) as xpool,
            tc.tile_pool(name="x0q", bufs=1) as x0q,
            tc.tile_pool(name="qkv", bufs=2) as qkv,
            tc.tile_pool(name="otp", bufs=5) as otp,
            tc.tile_pool(name="tmp", bufs=2) as tmp,
            tc.tile_pool(name="sqp", bufs=4) as sqp,
            tc.tile_pool(name="expool", bufs=8) as expool,
            tc.tile_pool(name="trp", bufs=4) as trp,
            tc.tile_pool(name="oop", bufs=10) as oop,
            tc.tile_pool(name="ps", bufs=1, space="PSUM") as ps,
        ):
            # ---- resident constants / weights ----
            wq_sb = wpool.tile([128, NKB, IL], F16, name="wq_sb")
            for kb2 in range(NKB // 2):
                nc.sync.dma_start(wq_sb[:, kb2 * 2:(kb2 + 1) * 2, :],
                                  Wq[:, kb2 * 2:(kb2 + 1) * 2, :])
            gq = wpool.tile([128, 1], F32, name="gq")
            nc.sync.dma_start(gq[:], qg[:])
            gk = wpool.tile([128, 1], F32, name="gk")
            nc.sync.dma_start(gk[:], kg[:])
            wk_sb = wpool.tile([128, NKB, IL], F16, name="wk_sb")
            wv_sb = wpool.tile([128, NKB, IL], F16, name="wv_sb")
            wo_sb = wpool.tile([128, HL, D], F16, name="wo_sb")
            ones_f32 = wpool.tile([128, 128], F32, name="ones_f32")
            nc.vector.memset(ones_f32[:], 1.0)
            ones16 = wpool.tile([128, 128], F16, name="ones16")
            nc.scalar.copy(ones16[:], ones_f32[:])
            eps_sb = wpool.tile([128, 1], F32, name="eps_sb")
            nc.vector.memset(eps_sb[:], EPS)
            # HAM warmup: the PE cold-throttles (K=4/8, ~2x slow) for the
            # first ~4us of matmuls.  Run dummy ones-matmuls while the
            # startup DMAs are in flight so real matmuls start at full
            # rate.  No readers -> the po-ring slots recycle instantly.
            for wu in range(60):
                wu_ps = ps.tile([128, 128], F32, tag="po", bufs=2,
                                name=f"wu{wu}")
                nc.tensor.matmul(wu_ps[:], ones16[:], ones16[:],
                                 start=True, stop=True)

            # ================= emission helpers =================
            # qkv tiles per batch (ring 2 => batches coexist)
            qkv_t = {}

            def ensure_qkv(b):
                if b not in qkv_t:
                    qkv_t[b] = (
                        qkv.tile([128, HL, S], F16, tag="qt", name=f"qt{b}"),
                        qkv.tile([128, HL, S], F16, tag="kt", name=f"kt{b}"),
                        qkv.tile([128, NKJ, IL], F16, tag="vt", name=f"vt{b}"),
                    )
                return qkv_t[b]

            xt_t = {}

            def emit_xt_dma(b, sc, split_rings=False, rings2=False):
                off = b * S + sc * SC
                if split_rings:
                    # startup chunk: eight independent eighth tiles on the
                    # scalar HWDGE ring (weights stream on the sync ring in
                    # parallel), so the first matmul waits only for the
                    # first 256KB piece instead of the whole 2MB chunk.
                    quads = []
                    for kb2 in range(NKB // 2):
                        q = x0q.tile([128, 2, SC], F16, tag=f"x0q{kb2}",
                                     name=f"x0q_{kb2}")
                        nc.scalar.dma_start(
                            q[:], xt_d[:, kb2 * 2:(kb2 + 1) * 2,
                                       off:off + SC])
                        quads.append(q)
                    xt_t[(b, sc)] = ("quad", quads)
                    return
                xt = xpool.tile([128, NKB, SC], F16, tag="xt",
                                name=f"xt{b}_{sc}")
                xt_t[(b, sc)] = xt
                for kb4 in range(NKB // 4):
                    # batch-0 chunks stream on both HWDGE rings (the ACT
                    # queue is ~40% busy in pure proj); batch-1 chunks
                    # stay on sync (ACT paces the merged attention).
                    eng = nc.scalar if (rings2 and kb4 < 2) else nc.sync
                    eng.dma_start(
                        xt[:, kb4 * 4:(kb4 + 1) * 4, :],
                        xt_d[:, kb4 * 4:(kb4 + 1) * 4, off:off + SC])

            def xt_slice(b, sc, kb, cols=None):
                xt = xt_t[(b, sc)]
                if isinstance(xt, tuple):
                    q = xt[1][kb // 2]
                    s = q[:, kb % 2, :]
                else:
                    s = xt[:, kb, :]
                return s if cols is None else s[:, cols]

            def emit_qk_pass(b, sc, pi, h):
                qt, kt, vt = ensure_qkv(b)
                w_sb, dstT, gam, pname = (
                    (wq_sb, qt, gq, "q") if pi == 0 else (wk_sb, kt, gk, "k"))
                ph = ps.tile([128, SC], F32, tag="ph", bufs=2,
                             name=f"ph_{pname}{b}_{sc}_{h}")
                for kb in range(NKB):
                    nc.tensor.matmul(
                        ph[:], w_sb[:, kb, h * 128:(h + 1) * 128],
                        xt_slice(b, sc, kb),
                        start=(kb == 0), stop=(kb == NKB - 1))
                sq = sqp.tile([128, SC], F16, tag="sq",
                              name=f"sq_{pname}{b}_{sc}_{h}")
                nc.scalar.activation(sq[:], ph[:], AF.Square)
                ssum = ps.tile([128, SC], F32, tag="sc", bufs=2,
                               name=f"ssum_{pname}{b}_{sc}_{h}")
                nc.tensor.matmul(ssum[:], ones16[:], sq[:],
                                 start=True, stop=True)
                # rstd = (mean(x^2)+eps)^-0.5 = exp(-0.5*ln(.))
                lg = tmp.tile([128, SC], F32, tag="lg",
                              name=f"lg_{pname}{b}_{sc}_{h}")
                nc.scalar.activation(lg[:], ssum[:], AF.Ln,
                                     bias=eps_sb[:], scale=1.0 / HEAD_DIM)
                rstd = tmp.tile([128, SC], F32, tag="rstd",
                                name=f"rstd_{pname}{b}_{sc}_{h}")
                nc.scalar.activation(rstd[:], lg[:], AF.Exp, scale=-0.5)
                nc.vector.scalar_tensor_tensor(
                    out=dstT[:, h, sc * SC:(sc + 1) * SC],
                    in0=ph[:], scalar=gam[:], in1=rstd[:],
                    op0=MUL, op1=MUL)

            def emit_v_pass(b, sc, j):
                _, _, vt = ensure_qkv(b)
                pv = ps.tile([128, IL], F32, tag="po", bufs=2,
                             name=f"pv{b}_{sc}_{j}")
                for kb in range(NKB):
                    nc.tensor.matmul(
                        pv[:], xt_slice(b, sc, kb,
                                        slice(j * 128, (j + 1) * 128)),
                        wv_sb[:, kb, :],
                        start=(kb == 0), stop=(kb == NKB - 1))
                nc.vector.tensor_copy(vt[:, sc * (SC // 128) + j, :], pv[:])

            def emit_wkwv_dma():
                for kb4 in range(NKB // 4):
                    nc.sync.dma_start(wk_sb[:, kb4 * 4:(kb4 + 1) * 4, :],
                                      Wk[:, kb4 * 4:(kb4 + 1) * 4, :])
                for kb4 in range(NKB // 4):
                    nc.sync.dma_start(wv_sb[:, kb4 * 4:(kb4 + 1) * 4, :],
                                      Wv[:, kb4 * 4:(kb4 + 1) * 4, :])

            def make_proj_units(b, first=False, defer_last=False):
                """Ordered emission units for batch b's projection phase.
                DMA for chunk sc+2 is emitted only after every chunk-sc
                unit (the xt ring-2 overwrite dependency tracks only
                already-emitted readers).  With defer_last, the final
                chunk's q and v passes are returned separately: they can
                run during batch b's OWN attention (vt[12..15] is first
                read ~7us into slot 0, qt cols 1536+ only at qb3), which
                moves PE work from the PE-bound merged phase into the
                ACT-bound final attention phase."""
                units = []
                units.append(lambda b=b: emit_xt_dma(b, 0, split_rings=first))
                if first:
                    units.append(emit_wkwv_dma)
                units.append(lambda b=b, f=first: emit_xt_dma(b, 1, rings2=f))
                late_v = []
                for sc in range(NSC):
                    defer = defer_last and sc == NSC - 1
                    for pi in range(2):
                        for h in range(HL):
                            units.append(
                                lambda b=b, sc=sc, pi=pi, h=h:
                                emit_qk_pass(b, sc, pi, h))
                    for j in range(SC // 128):
                        u = (lambda b=b, sc=sc, j=j: emit_v_pass(b, sc, j))
                        (late_v if defer else units).append(u)
                    if sc + 2 < NSC:
                        units.append(
                            lambda b=b, sc=sc, f=first:
                            emit_xt_dma(b, sc + 2, rings2=f))
                # vt[12..15] is read from kj 12 of the very first
                # attention slot, so the deferred v passes pump early.
                return units, late_v

            # ---- attention software-pipeline state ----
            pend_tail = [None]
            pend_po = [None]
            cur_attn_b = [0]
            oo_ctr = [0]
            po_units = []
            proj_units = []
            late_units = []

            def tail_step(step):
                """One stage of the previous slot's softmax tail.
                step 1: den matmul; 2: Ln; 3: recip exp; 4: ot mul."""
                if pend_tail[0] is None:
                    return
                av, roots, ot_t, h, bb, qq, st = pend_tail[0]
                if step == 1:
                    den = ps.tile([128, 512], F32, tag="po", bufs=2,
                                  name=f"den{bb}_{qq}_{h}")
                    nr = len(roots)
                    for i, rt in enumerate(roots):
                        nc.tensor.matmul(den[:], ones16[:], rt[:],
                                         start=(i == 0), stop=(i == nr - 1))
                    st["den"] = den
                elif step == 2:
                    lden = tmp.tile([128, 512], F32, tag="lden",
                                    name=f"lden{bb}_{qq}_{h}")
                    nc.scalar.activation(lden[:], st["den"][:], AF.Ln)
                    st["lden"] = lden
                elif step == 3:
                    r = tmp.tile([128, 512], F32, tag="r",
                                 name=f"r{bb}_{qq}_{h}")
                    nc.scalar.activation(r[:], st["lden"][:], AF.Exp,
                                         scale=-1.0)
                    st["r"] = r
                elif step == 4:
                    nc.vector.tensor_mul(ot_t[:, h, :], av[:], st["r"][:])
                    pend_tail[0] = None

            def po_load_units():
                if pend_po[0] is None:
                    return
                bb, qq, ot_t = pend_po[0]
                pend_po[0] = None
                for qs in range(4):
                    for dc in range(D // 512):
                        po_units.append((bb, qq, ot_t, qs, dc))

            def po_step(n=1, split_rings=False):
                """Emit n output-projection units (2 matmuls + copy + DMA).
                split_rings alternates the out-DMA between the sync and
                scalar HWDGE rings -- only safe when the scalar (ACT)
                queue has no pending exp work, i.e. at the final flush."""
                for i_ in range(n):
                    if not po_units:
                        return
                    bb, qq, ot_t, qs, dc = po_units.pop(0)
                    qi = qq * 4 + qs
                    po = ps.tile([128, 512], F32, tag="po", bufs=2,
                                 name=f"po{bb}_{qi}_{dc}")
                    for h in range(HL):
                        nc.tensor.matmul(
                            po[:], ot_t[:, h, qs * 128:(qs + 1) * 128],
                            wo_sb[:, h, dc * 512:(dc + 1) * 512],
                            start=(h == 0), stop=(h == HL - 1))
                    oo = oop.tile([128, 512], F16, tag="oo",
                                  name=f"oo{bb}_{qi}_{dc}")
                    oo_ctr[0] += 1
                    if (split_rings and i_ % 2 == 1) or (
                            bb == B - 1 and qq == NQB - 2
                            and oo_ctr[0] % 2 == 1):
                        # ACT absorbs half the psum->fp16 copies where it
                        # has slack: the final flush (exp stream done) and
                        # the last qb's slots (exp stream tapering, DVE
                        # paces with tree+ot+copies).
                        nc.scalar.copy(oo[:], po[:])
                    else:
                        nc.vector.tensor_copy(oo[:], po[:])
                    eng = nc.scalar if (split_rings and i_ % 2 == 1) else nc.sync
                    eng.dma_start(
                        out[bb * S + qi * 128: bb * S + (qi + 1) * 128,
                            dc * 512:(dc + 1) * 512], oo[:])

            def proj_step(n=1):
                for _ in range(n):
                    if not proj_units:
                        return
                    proj_units.pop(0)()

            def late_step(n=1):
                for _ in range(n):
                    if not late_units:
                        return
                    late_units.pop(0)()

            # ================= main schedule =================
            # batch-0 projection runs pure (nothing to overlap with).
            u0, _ = make_proj_units(0, first=True)
            for u in u0:
                u()
            nc.sync.dma_start(wo_sb[:], Wo[:])
            u1, l1 = make_proj_units(1, defer_last=True)
            proj_units.extend(u1)
            late_units.extend(l1)

            for b in range(B):
                cur_attn_b[0] = b
                if b > 0:
                    # safety: attention for b reads qt/kt/vt(b) -- every
                    # proj unit must be emitted by now (normally the
                    # pump points have already drained the list).
                    proj_step(len(proj_units))
                qt, kt, vt = ensure_qkv(b)
                for qb in range(NQB):
                    ot_qb = otp.tile([128, HL, 512], F16, tag="ot",
                                     name=f"ot{b}_{qb}")
                    for h in range(HL):
                        if h == HL - 1:
                            po_load_units()
                        av = ps.tile([128, 512], F32, tag="av", bufs=2,
                                     name=f"av{b}_{h}_{qb}")
                        roots = []
                        lvl_pend = [[] for _ in range(TREE_LVLS)]
                        exs = []

                        def av_and_tree(j, h=h, av=av, exs=exs,
                                        lvl_pend=lvl_pend, roots=roots,
                                        b=b, qb=qb):
                            # av matmul + pair-sum tree for key block j;
                            # runs one kj behind the score matmuls so the
                            # in-order PE never stalls on exp(j): the
                            # sc->exp->av latency chain paced the pure
                            # attention phase at ~19us/slot.
                            nc.tensor.matmul(
                                av[:], vt[:, j, h * 128:(h + 1) * 128],
                                exs[j][:],
                                start=(j == 0), stop=(j == NKJ - 1))
                            node = exs[j]
                            for lv in range(TREE_LVLS):
                                lvl_pend[lv].append(node)
                                if len(lvl_pend[lv]) < 2:
                                    node = None
                                    break
                                a0, a1 = lvl_pend[lv]
                                lvl_pend[lv] = []
                                t = trp.tile([128, 512], F16, tag=f"tr{lv}",
                                             bufs=(4 if lv < 3 else 3),
                                             name=f"tr{lv}_{b}_{h}_{qb}_{j}")
                                nc.vector.tensor_add(t[:], a0[:], a1[:])
                                node = t
                            if node is not None:
                                roots.append(node)

                        next_av = [0]
                        for kj in range(NKJ):
                            # once the proj-unit list is empty the ph ring
                            # has no future users: alternate the score
                            # psum between sc and ph tags (effective ring
                            # 4) and deepen the lookahead -- this covers
                            # the final batch AND the tail of the merged
                            # phase, whose proj units run out early.
                            free_ph = (b == B - 1) or not proj_units
                            stag = "ph" if (free_ph and kj % 2 == 1) \
                                else "sc"
                            sc_ps = ps.tile([128, 512], F32, tag=stag,
                                            bufs=2,
                                            name=f"sc{b}_{h}_{qb}_{kj}")
                            nc.tensor.matmul(
                                sc_ps[:], kt[:, h, kj * 128:(kj + 1) * 128],
                                qt[:, h, qb * 512:(qb + 1) * 512],
                                start=True, stop=True)
                            ex = expool.tile([128, 512], F16, tag="ex",
                                             name=f"ex{b}_{h}_{qb}_{kj}")
                            nc.scalar.activation(ex[:], sc_ps[:], AF.Exp,
                                                 scale=scale)
                            exs.append(ex)
                            # lookahead depth 2 when the ph alternation is
                            # active (effective sc ring 4), else 1.
                            lag = 2 if free_ph else 1
                            if kj == 0:
                                po_step(1)
                            while next_av[0] <= kj - lag:
                                av_and_tree(next_av[0])
                                next_av[0] += 1
                            # pump slack work into the PE's exp-wait gaps
                            # (the PE executes matmuls in program order).
                            if kj in (1, 2, 3, 4):
                                tail_step(kj)
                            if kj % 2 == 1:
                                po_step(1)
                            if kj in PROJ_PUMP_KJ:
                                proj_step(1)
                            if b == B - 1 and kj in (2, 4, 6, 8):
                                # deferred b1 v passes: vt[12..15] is
                                # first read at kj 12 of this slot.
                                late_step(1)
                        while next_av[0] < NKJ:
                            av_and_tree(next_av[0])
                            next_av[0] += 1
                        pend_tail[0] = (av, roots, ot_qb, h, b, qb, {})
                    # drain toward 2 (keep kj0 gap-fillers) but cap the
                    # burst so the ACT exp pipeline never starves behind
                    # a long run of po matmuls.
                    po_step(min(max(0, len(po_units) - 2), 8))
                    pend_po[0] = (b, qb, ot_qb)

            # final flush
            proj_step(len(proj_units))
            late_step(len(late_units))
            for stp in (1, 2, 3, 4):
                tail_step(stp)
            po_load_units()
            po_step(len(po_units), split_rings=True)

    if split:
        _split_waits(nc)
    return nc


def _prep_in_maps(inputs, B, S, D, HL):
    """Shard + pre-tile the full inputs for the 8 cores (all fp16)."""
    BS = B * S
    NKB = D // 128
    IL = HL * HEAD_DIM
    x = np.asarray(inputs["x"], dtype=np.float32)
    Wq = np.asarray(inputs["Wq"], dtype=np.float32)
    Wk = np.asarray(inputs["Wk"], dtype=np.float32)
    Wv = np.asarray(inputs["Wv"], dtype=np.float32)
    Wo = np.asarray(inputs["Wo"], dtype=np.float32)
    qg = np.ascontiguousarray(
        np.asarray(inputs["q_gamma"], dtype=np.float32).reshape(128, 1))
    kg = np.ascontiguousarray(
        np.asarray(inputs["k_gamma"], dtype=np.float32).reshape(128, 1))
    # x[bs, kb*128+p] -> [p, kb, bs]
    xtld = np.ascontiguousarray(
        x.reshape(BS, NKB, 128).transpose(2, 1, 0).astype(np.float16))
    in_maps = []
    for c in range(N_CORES):
        cs = slice(c * IL, (c + 1) * IL)
        # W[kb*128+p, n] -> [p, kb, n]
        wq_t = np.ascontiguousarray(
            Wq[:, cs].reshape(NKB, 128, IL).transpose(1, 0, 2).astype(np.float16))
        wk_t = np.ascontiguousarray(
            Wk[:, cs].reshape(NKB, 128, IL).transpose(1, 0, 2).astype(np.float16))
        wv_t = np.ascontiguousarray(
            Wv[:, cs].reshape(NKB, 128, IL).transpose(1, 0, 2).astype(np.float16))
        # Wo[h*128+p, :] -> [p, h, :]
        wo_t = np.ascontiguousarray(
            Wo[cs, :].reshape(HL, 128, D).transpose(1, 0, 2).astype(np.float16))
        in_maps.append({
            "xt_d": xtld,
            "Wq": wq_t,
            "Wk": wk_t,
            "Wv": wv_t,
            "Wo": wo_t,
            "qg": qg,
            "kg": kg,
        })
    return in_maps


_NC_CACHE = {}


def run_cores(inputs, trace=False):
    """Build (cached), shard, run on 8 cores; returns (full_out, results)."""
    from concourse.bass_utils import run_bass_kernel_spmd

    x = np.asarray(inputs["x"])
    B, S, D = x.shape
    HL = N_HEADS // N_CORES
    key = (B, S, D, HL)
    if key not in _NC_CACHE:
        _NC_CACHE[key] = build_nc(B, S, D, HL)
    nc = _NC_CACHE[key]
    in_maps = _prep_in_maps(inputs, B, S, D, HL)
    res = run_bass_kernel_spmd(nc, in_maps, list(range(N_CORES)), trace=trace)
    acc = res.results[0]["out"].astype(np.float32)
    for c in range(1, N_CORES):
        acc = acc + res.results[c]["out"].astype(np.float32)
    return acc.reshape(B, S, D), res


def kernel(**inputs) -> np.ndarray:
    return run_cores(inputs, trace=False)[0]
